# revision 30
# baseline (speedup 1.0000x reference)
"""Trainium2 Bass kernel for nn_CNL_5 (5-scale context non-local block).

Sharding: 8 cores = 4 samples x 2 query-subsets. Local query column order is
L = 64*j + q  (q = z-conv input channel = n//36-block, j = n%18), chosen so the
xbar DMA transpose (out[p,b,c] = in[c,128b+p]) directly yields the z-conv
operand x[q, pixel] with j-parity split across partition halves. outT is
padded to 640-col halves so each query-half transposes independently.

BN batch stats travel as per-channel (s1, s2) quadratic forms [128,20] through
one small AllGather; heavy math is fp16/bf16 on the PE at 1 cyc/row.
"""
import numpy as np
import ml_dtypes
from contextlib import ExitStack

import concourse.bass as bass
import concourse.bacc as bacc
import concourse.tile as tile
from concourse import mybir
from concourse import bass_utils
from concourse.alu_op_type import AluOpType

F32 = mybir.dt.float32
F16 = mybir.dt.float16
BF16 = mybir.dt.bfloat16
AFT = mybir.ActivationFunctionType
AXX = mybir.AxisListType.X

NCORES = 8
CP = 256
QL = 1152
CR = [64, 256, 512, 1024, 2048]
MS = [2304, 2304, 576, 144, 36]
CSH = [0.0, 10.0, 15.0, 25.0, 40.0]
EPS = 1e-5
NPIX = 4 * 2304.0
SCHED = [4, 2, 3, 1, 0]
DEBUG = False

_CACHED = {}


def mtiles(M):
    out, off = [], 0
    while off < M:
        w = min(128, M - off)
        out.append((off, w))
        off += w
    return out


def chunks512(N):
    out, off = [], 0
    while off < N:
        w = min(512, N - off)
        out.append((off, w))
        off += w
    return out


def build():
    nc = bacc.Bacc("TRN2", target_bir_lowering=False, debug=False,
                   num_devices=NCORES)
    KT = [len(mtiles(c)) for c in CR]
    persp_d = nc.dram_tensor("persp", [128, 2 * QL], F16, kind="ExternalInput").ap()
    twt_d = nc.dram_tensor("twt", [128, 128], F16, kind="ExternalInput").ap()
    resp_d = [nc.dram_tensor(f"resp{s}", [min(CR[s], 128), KT[s] * MS[s]], F16,
                             kind="ExternalInput").ap() for s in range(5)]
    pwt_d = nc.dram_tensor("pwt", [128, 64 * sum(KT)], F16, kind="ExternalInput").ap()
    gwt_d = nc.dram_tensor("gwt", [128, 64 * sum(KT)], F16, kind="ExternalInput").ap()
    zwt16_d = nc.dram_tensor("zwt16", [128, CP], F16, kind="ExternalInput").ap()
    zwtf_d = nc.dram_tensor("zwtf", [64, CP], F32, kind="ExternalInput").ap()
    zw65_d = nc.dram_tensor("zw65", [128, 650], F32, kind="ExternalInput").ap()
    gmp_d = nc.dram_tensor("gmp", [128, 13], F32, kind="ExternalInput").ap()
    i128_d = nc.dram_tensor("i128", [128, 128], F16, kind="ExternalInput").ap()
    out_d = nc.dram_tensor("out", [128, 2 * QL], F32, kind="ExternalOutput").ap()
    dbg = {}
    if DEBUG:
        for nm, shp, dt in [("d_t16", [64, QL], F16), ("d_p16", [64, MS[2]], F16),
                            ("d_outT", [64, 1280], F16), ("d_xw0", [128, 320], F16),
                            ("d_xw1", [128, 320], F16), ("d_Gcat", [64, 325], F32),
                            ("d_arin", [128, 20], F32), ("d_stats", [128, 20], F32),
                            ("d_a16", [128, 10], F16), ("d_W0", [128, 256], F16),
                            ("d_opn", [64, 576], F32), ("d_opd", [64, 576], F32),
                            ("d_rc", [64, 576], F32), ("d_et", [128, 576], F16),
                            ("d_gall", [128, 640], F32)]:
            dbg[nm] = nc.dram_tensor(nm, shp, dt, kind="ExternalOutput").ap()

    with tile.TileContext(nc) as tc, ExitStack() as ctx:
        sb = ctx.enter_context(tc.tile_pool(name="sb", bufs=1))
        p2 = ctx.enter_context(tc.tile_pool(name="p2", bufs=2))
        et3 = ctx.enter_context(tc.tile_pool(name="et3", bufs=3))
        p16p = ctx.enter_context(tc.tile_pool(name="p16p", bufs=3))
        dram = ctx.enter_context(tc.tile_pool(name="dram", bufs=1, space="DRAM"))
        psc = ctx.enter_context(tc.tile_pool(name="psc", bufs=2, space="PSUM"))
        pop = ctx.enter_context(tc.tile_pool(name="pop", bufs=1, space="PSUM"))
        pcv = ctx.enter_context(tc.tile_pool(name="pcv", bufs=1, space="PSUM"))
        pgx = ctx.enter_context(tc.tile_pool(name="pgx", bufs=1, space="PSUM"))

        # ---------------- static loads ----------------
        twt_sb = sb.tile([128, 128], F16, tag="twt", name="twt")
        nc.sync.dma_start(twt_sb[:], twt_d)
        persp_sb = sb.tile([128, 2 * QL], F16, tag="persp", name="persp")
        nc.sync.dma_start(persp_sb[:], persp_d)
        pwt_sb = sb.tile([128, 64 * sum(KT)], F16, tag="pwt", name="pwt")
        nc.sync.dma_start(pwt_sb[:], pwt_d)
        resp_sb = [None] * 5
        s0_ = SCHED[0]
        r = sb.tile([min(CR[s0_], 128), KT[s0_] * MS[s0_]], F16,
                    tag=f"resp{s0_}", name=f"resp{s0_}")
        nc.sync.dma_start(r[:], resp_d[s0_])
        resp_sb[s0_] = r
        gwt_sb = sb.tile([128, 64 * sum(KT)], F16, tag="gwt", name="gwt")
        nc.sync.dma_start(gwt_sb[:], gwt_d)
        for s in SCHED[1:]:
            r = sb.tile([min(CR[s], 128), KT[s] * MS[s]], F16,
                        tag=f"resp{s}", name=f"resp{s}")
            nc.sync.dma_start(r[:], resp_d[s])
            resp_sb[s] = r
        KOFF = [64 * sum(KT[:s]) for s in range(5)]
        zwt16_sb = sb.tile([128, CP], F16, tag="zwt16", name="zwt16")
        nc.sync.dma_start(zwt16_sb[:], zwt16_d)
        zwtf_sb = sb.tile([64, CP], F32, tag="zwtf", name="zwtf")
        nc.sync.dma_start(zwtf_sb[:], zwtf_d)
        zw65g_sb = sb.tile([128, 650], F32, tag="zw65", name="zw65")
        nc.sync.dma_start(zw65g_sb[:], zw65_d)
        gmp_sb = sb.tile([128, 13], F32, tag="gmp", name="gmp")
        nc.sync.dma_start(gmp_sb[:], gmp_d)
        i128_sb = sb.tile([128, 128], F16, tag="i128", name="i128")
        nc.sync.dma_start(i128_sb[:], i128_d)
        bias_sb = []
        for s in range(5):
            bt = sb.tile([128, 1], F32, tag=f"bias{s}", name=f"bias{s}")
            nc.vector.memset(bt[:], -CSH[s])
            bias_sb.append(bt)
        ones128 = sb.tile([1, 128], F16, tag="ones128", name="ones128")
        nc.vector.memset(ones128[:], 1.0)
        onesc = sb.tile([64, 1], F16, tag="onesc", name="onesc")
        nc.vector.memset(onesc[:], 1.0)
        g_all = [sb.tile([128, 128 * 18], BF16, tag=f"gall{i}", name=f"gall{i}")
                 for i in range(3)]
        for i in range(3):
            nc.gpsimd.memset(
                g_all[i][:].rearrange("p (k c) -> p k c", c=128)[:, :, 64:128], 1.0)
        # outT ring: pre-zero the 64-col pads of both ring slots
        oT = [p2.tile([64, 1280], F16, tag="outT", name=f"outTz{i}")
              for i in range(2)]
        for i in range(2):
            for h in range(2):
                nc.gpsimd.memset(oT[i][:, 640 * h + 576:640 * h + 640], 0.0)

        # ---------------- t conv: t16 [64, QL] ----------------
        t16 = sb.tile([64, QL], F16, tag="t16", name="t16")
        for off, w in chunks512(QL):
            tp = pgx.tile([128, 512], F32, tag="gx", name="gx")
            for kk in range(2):
                nc.tensor.matmul(tp[0:64, 0:w], twt_sb[:, 64 * kk:64 * kk + 64],
                                 persp_sb[:, QL * kk + off:QL * kk + off + w],
                                 start=(kk == 0), stop=(kk == 1))
            nc.vector.tensor_copy(t16[:, off:off + w], tp[0:64, 0:w])

        # ---------------- per-scale p/g convs (as deferrable units) --------
        p16_sb, xw_sb = {}, {}

        def conv_units(s, evac_eng):
            nct = mtiles(CR[s])
            p16 = p16p.tile([64, MS[s]], F16, tag="p16", name=f"p16_{s}")
            p16_sb[s] = p16
            units = []

            def p_unit(off, w):
                def emit():
                    pp = pcv.tile([128, 512], F32, tag="cv", name="cv")
                    for kk, (co, cw) in enumerate(nct):
                        nc.tensor.matmul(
                            pp[0:64, 0:w],
                            pwt_sb[0:cw, KOFF[s] + 64 * kk:KOFF[s] + 64 * kk + 64],
                            resp_sb[s][0:cw, MS[s] * kk + off:MS[s] * kk + off + w],
                            start=(kk == 0), stop=(kk == len(nct) - 1))
                    evac_eng.tensor_copy(p16[:, off:off + w], pp[0:64, 0:w])
                return emit

            def g_unit(b0, batch):
                def emit():
                    ga = g_all[SCHED.index(s) % 3]
                    gp = pcv.tile([128, 512], F32, tag="cv", name="cv")
                    for k, (moff, mw) in enumerate(batch):
                        for kk, (co, cw) in enumerate(nct):
                            nc.tensor.matmul(
                                gp[0:mw, 64 * k:64 * k + 64],
                                resp_sb[s][0:cw, MS[s] * kk + moff:MS[s] * kk + moff + mw],
                                gwt_sb[0:cw, KOFF[s] + 64 * kk:KOFF[s] + 64 * kk + 64],
                                start=(kk == 0), stop=(kk == len(nct) - 1))
                    dst = ga[:].rearrange("p (k c) -> p k c", c=128)[
                        :, b0:b0 + len(batch), 0:64]
                    src = gp[:].rearrange("p (k c) -> p k c", c=64)[
                        :, 0:len(batch), :]
                    evac_eng.tensor_copy(dst, src)
                return emit

            for off, w in chunks512(MS[s]):
                units.append(p_unit(off, w))
            mts = mtiles(MS[s])
            for b0 in range(0, len(mts), 8):
                units.append(g_unit(b0, mts[b0:b0 + 8]))
            return units

        for u in conv_units(SCHED[0], nc.vector):
            u()
        pending = conv_units(SCHED[1], nc.vector)

        # ---------------- attention per scale ----------------
        arin_sb = sb.tile([128, 20], F32, tag="arin", name="arin")
        G_cat = sb.tile([64, 325], F32, tag="G_cat", name="G_cat")
        for si, s in enumerate(SCHED):
            mts = mtiles(MS[s])
            ga = g_all[si % 3]
            while pending:
                pending.pop(0)()
            if si + 2 < 5:
                pending += conv_units(SCHED[si + 2], nc.vector)
            outT = p2.tile([64, 1280], F16, tag="outT", name=f"outT{s}")
            gm = pgx.tile([128, 512], F32, tag="gx", name="gx")
            for h in range(2):
                op = pop.tile([128, 576], F32, tag="op", name="op")
                for k, (moff, mw) in enumerate(mts):
                    sc = psc.tile([128, 576], F32, tag="sc", name="sc")
                    for co, cw in ((0, 512), (512, 64)):
                        nc.tensor.matmul(
                            sc[0:mw, co:co + cw],
                            p16_sb[s][:, moff:moff + mw],
                            t16[:, 576 * h + co:576 * h + co + cw],
                            start=True, stop=True)
                    et = et3.tile([128, 576], BF16, tag="et", name="et")
                    nc.scalar.activation(et[0:mw, :], sc[0:mw, :], AFT.Exp,
                                         bias=bias_sb[s][0:mw, :])
                    if DEBUG and si == 0 and h == 0 and k == 0:
                        de = sb.tile([128, 576], F16, tag="dbg3", name="dbg3")
                        nc.vector.tensor_copy(de[:], et[:])
                        nc.sync.dma_start(dbg["d_et"], de[:])
                    for co, cw in ((0, 512), (512, 64)):
                        nc.tensor.matmul(
                            op[:, co:co + cw],
                            ga[0:mw, 128 * k:128 * k + 128],
                            et[0:mw, co:co + cw],
                            start=(k == 0), stop=(k == len(mts) - 1))
                    if pending and (k % 2 == 1 or len(mts) < 10):
                        pending.pop(0)()
                rc = p2.tile([64, 576], F32, tag="rc", name="rc")
                if DEBUG and si == 0 and h == 1:
                    dn = sb.tile([64, 576], F32, tag="dbg1", name="dbg1")
                    nc.vector.tensor_copy(dn[:], op[0:64, :])
                    nc.sync.dma_start(dbg["d_opn"], dn[:])
                    dd = sb.tile([64, 576], F32, tag="dbg2", name="dbg2")
                    nc.vector.tensor_copy(dd[:], op[64:128, :])
                    nc.sync.dma_start(dbg["d_opd"], dd[:])
                nc.vector.reciprocal(rc[:], op[64:128, :])
                if DEBUG and si == 0 and h == 1:
                    nc.sync.dma_start(dbg["d_rc"], rc[:])
                nc.vector.tensor_tensor(outT[:, 640 * h:640 * h + 576],
                                        op[0:64, :], rc[:], op=AluOpType.mult)
                # per-half xbar transpose + svec + gram
                xw = sb.tile([128, 5 * 64], F16, tag=f"xw{s}_{h}",
                             name=f"xw{s}_{h}")
                xw_sb[(s, h)] = xw
                nc.sync.dma_start_transpose(
                    xw[:].rearrange("p (b c) -> p b c", c=64),
                    outT[:, 640 * h:640 * h + 640])
                for j in range(9):
                    nc.tensor.matmul(
                        gm[0:64, 0:64],
                        outT[:, 640 * h + 64 * j:640 * h + 64 * j + 64],
                        outT[:, 640 * h + 64 * j:640 * h + 64 * j + 64],
                        start=(h == 0 and j == 0), stop=(h == 1 and j == 8))
            if DEBUG and si == 0:
                nc.sync.dma_start(dbg["d_outT"], outT[:])
            # order in one bank: gram -> G-copy -> svec-sum -> svrow-copy ->
            # transpose -> G-col copy (chained via overlapping regions, since
            # a matmul 'start' resets the whole bank)
            nc.vector.tensor_copy(G_cat[:, 65 * si:65 * si + 64],
                                  gm[0:64, 0:64])
            for h in range(2):
                for j in range(9):
                    nc.tensor.matmul(
                        gm[0:1, 0:64], onesc[:],
                        outT[:, 640 * h + 64 * j:640 * h + 64 * j + 64],
                        start=(h == 0 and j == 0), stop=(h == 1 and j == 8))
            svrow = p2.tile([1, 64], F16, tag="svrow", name="svrow")
            nc.vector.tensor_copy(svrow[:], gm[0:1, 0:64])
            nc.tensor.matmul(gm[0:64, 0:1], svrow[:], onesc[0:1, 0:1],
                             start=True, stop=True)
            nc.vector.tensor_copy(G_cat[:, 65 * si + 64:65 * si + 65],
                                  gm[0:64, 0:1])
            if si == 3:
                # BN partials for first 4 scheduled scales (off critical path)
                for t in range(2):
                    zgp = (pgx if t == 0 else pcv).tile(
                        [128, 512], F32, tag="gx" if t == 0 else "cv",
                        name="zgP")
                    nc.tensor.matmul(zgp[:, 0:260],
                                     zwtf_sb[:, 128 * t:128 * t + 128],
                                     G_cat[:, 0:260], start=True, stop=True)
                    zzp = p2.tile([128, 260], F32, tag="zz", name="zz")
                    nc.vector.tensor_tensor(
                        zzp[:], zgp[:, 0:260],
                        zw65g_sb[:, 325 * t:325 * t + 260],
                        op=AluOpType.mult)
                    nc.vector.tensor_reduce(
                        arin_sb[:, 10 * t:10 * t + 4],
                        zzp[:].rearrange("p (s c) -> p s c", c=65), AXX,
                        AluOpType.add)
                    nc.vector.tensor_copy(
                        arin_sb[:, 10 * t + 5:10 * t + 9],
                        bass.AP(tensor=zgp[:].tensor,
                                offset=zgp[:].offset + 64,
                                ap=[[zgp[:].ap[0][0], 128], [65, 4]]))

        if DEBUG:
            dg = sb.tile([128, 640], F32, tag="dbg4", name="dbg4")
            nc.vector.tensor_copy(dg[:], g_all[0][:, 0:640])
            nc.sync.dma_start(dbg["d_gall"], dg[:])
            nc.sync.dma_start(dbg["d_t16"], t16[:])
            nc.sync.dma_start(dbg["d_p16"], p16_sb[2][:])
            nc.sync.dma_start(dbg["d_Gcat"], G_cat[:])
            nc.sync.dma_start(dbg["d_xw0"], xw_sb[(2, 0)][:])
            nc.sync.dma_start(dbg["d_xw1"], xw_sb[(2, 1)][:])
        # last scheduled scale's BN stats (cols 260:325 of G_cat)
        for t in range(2):
            zg = (pgx if t == 0 else pcv).tile(
                [128, 512], F32, tag="gx" if t == 0 else "cv", name="zgL")
            nc.tensor.matmul(zg[:, 0:65], zwtf_sb[:, 128 * t:128 * t + 128],
                             G_cat[:, 260:325], start=True, stop=True)
            zzl = p2.tile([128, 65], F32, tag="zzL", name="zzL")
            nc.vector.tensor_tensor(zzl[:], zg[:, 0:65],
                                    zw65g_sb[:, 325 * t + 260:325 * t + 325],
                                    op=AluOpType.mult)
            nc.vector.tensor_reduce(arin_sb[:, 10 * t + 4:10 * t + 5],
                                    zzl[:], AXX, AluOpType.add)
            nc.vector.tensor_copy(arin_sb[:, 10 * t + 9:10 * t + 10],
                                  zg[:, 64:65])

        if DEBUG:
            nc.sync.dma_start(dbg["d_arin"], arin_sb[:])
        # ---------------- stats AllGather ----------------
        arin = dram.tile([128, 20], F32, name="arin_d")
        arout = dram.tile([128 * NCORES, 20], F32, name="arout_d")
        nc.sync.dma_start(arin[:], arin_sb[:])
        nc.gpsimd.collective_compute(
            "AllGather", AluOpType.bypass,
            replica_groups=[list(range(NCORES))],
            ins=[arin.opt()], outs=[arout.opt()])
        gath = sb.tile([128, 160], F32, tag="gath", name="gath")
        src = bass.AP(tensor=arout[:].tensor, offset=arout[:].offset,
                      ap=[[20, 128], [2560, 8], [1, 20]])
        nc.sync.dma_start(gath[:], src)
        stats = sb.tile([128, 20], F32, tag="stats", name="stats")
        nc.vector.tensor_reduce(
            stats[:],
            bass.AP(tensor=gath[:].tensor, offset=gath[:].offset,
                    ap=[[gath[:].ap[0][0], 128], [1, 20], [20, 8]]),
            AXX, AluOpType.add)

        if DEBUG:
            nc.sync.dma_start(dbg["d_stats"], stats[:])
        # ---------------- BN coefficients (SCHED order) ----------------
        a16 = sb.tile([128, 10], F16, tag="a16", name="a16")
        bacc_t = [sb.tile([128, 1], F32, tag=f"bacc{t}", name=f"bacc{t}")
                  for t in range(2)]
        for t in range(2):
            s2v = stats[:, 10 * t:10 * t + 5]
            s1v = stats[:, 10 * t + 5:10 * t + 10]
            mean = p2.tile([128, 5], F32, tag="mean", name="mean")
            nc.vector.tensor_scalar_mul(mean[:], s1v, 1.0 / NPIX)
            m2 = p2.tile([128, 5], F32, tag="m2", name="m2")
            nc.vector.tensor_tensor(m2[:], mean[:], mean[:], op=AluOpType.mult)
            var = p2.tile([128, 5], F32, tag="var", name="var")
            nc.vector.scalar_tensor_tensor(var[:], s2v, 1.0 / NPIX, m2[:],
                                           op0=AluOpType.mult,
                                           op1=AluOpType.subtract)
            sq = p2.tile([128, 5], F32, tag="sq", name="sq")
            nc.scalar.activation(sq[:], var[:], AFT.Sqrt,
                                 bias=gmp_sb[:, 12:13])
            rinv = p2.tile([128, 5], F32, tag="rinv", name="rinv")
            nc.vector.reciprocal_approx_fast(rinv[:], sq[:])
            af = p2.tile([128, 5], F32, tag="af", name="af")
            nc.vector.tensor_tensor(af[:], rinv[:], gmp_sb[:, 5 * t:5 * t + 5],
                                    op=AluOpType.mult)
            nc.vector.tensor_copy(a16[:, 5 * t:5 * t + 5], af[:])
            tmb = p2.tile([128, 5], F32, tag="tmb", name="tmb")
            nc.vector.tensor_tensor(tmb[:], af[:], mean[:], op=AluOpType.mult)
            tmbr = p2.tile([128, 1], F32, tag="tmbr", name="tmbr")
            nc.vector.tensor_reduce(tmbr[:], tmb[:], AXX, AluOpType.add)
            nc.vector.tensor_tensor(bacc_t[t][:], gmp_sb[:, 10 + t:11 + t],
                                    tmbr[:], op=AluOpType.subtract)
        # a5cat rows via PE transposes: one accumulation group per bank
        # (start zeroes the bank; disjoint-column matmuls add into zeros),
        # then 3 wide copies instead of 10 narrow ones
        a5cat = sb.tile([1, 1280], F16, tag="a5cat", name="a5cat")
        banks = [(pgx, "gx", 0, 4), (pcv, "cv", 4, 8), (psc, "sc", 8, 10)]
        for pool, tag, i0, i1 in banks:
            atp = pool.tile([128, 512], F32, tag=tag, name="tp")
            for ii in range(i0, i1):
                si, t = ii // 2, ii % 2
                nc.tensor.matmul(
                    atp[0:1, 128 * (ii - i0):128 * (ii - i0) + 128],
                    a16[:, 5 * t + si:5 * t + si + 1],
                    i128_sb[:], start=(ii == i0), stop=(ii == i1 - 1))
            nc.vector.tensor_copy(a5cat[0:1, 128 * i0:128 * i1],
                                  atp[0:1, 0:128 * (i1 - i0)])
        W_sb = []
        for si in range(5):
            abp = (pcv if si % 2 else pgx).tile(
                [128, 512], F32, tag="cv" if si % 2 else "gx", name="ab")
            nc.tensor.matmul(abp[:, 0:256], ones128[:],
                             a5cat[0:1, 256 * si:256 * si + 256],
                             start=True, stop=True)
            W = sb.tile([128, 256], F16, tag=f"W{si}", name=f"W{si}")
            nc.vector.tensor_tensor(W[:], zwt16_sb[:], abp[:, 0:256],
                                    op=AluOpType.mult)
            W_sb.append(W)

        if DEBUG:
            nc.sync.dma_start(dbg["d_a16"], a16[:])
            nc.sync.dma_start(dbg["d_W0"], W_sb[0][:])
        # ---------------- final matmul + store ----------------
        for t in range(2):
            out_sb = sb.tile([128, QL], F32, tag=f"osb{t}", name=f"osb{t}")
            for h in range(2):
                for par in range(2):
                    nb = 5 if par == 0 else 4
                    fp = psc.tile([128, 576], F32, tag="sc", name="sc")
                    for si in range(5):
                        nc.tensor.matmul(
                            fp[:, 0:64 * nb],
                            W_sb[si][64 * par:64 * par + 64,
                                     128 * t:128 * t + 128],
                            xw_sb[(SCHED[si], h)][64 * par:64 * par + 64,
                                                  0:64 * nb],
                            start=(si == 0), stop=(si == 4))
                    dst = bass.AP(
                        tensor=out_sb[:].tensor,
                        offset=out_sb[:].offset + 64 * (9 * h + par),
                        ap=[[out_sb[:].ap[0][0], 128], [128, nb], [1, 64]])
                    nc.vector.tensor_scalar_add(
                        dst,
                        fp[:].rearrange("p (b c) -> p b c", c=64)[:, 0:nb, :],
                        bacc_t[t][:])
                nc.sync.dma_start(
                    out_d[:, QL * t + 576 * h:QL * t + 576 * h + 576],
                    out_sb[:, 576 * h:576 * h + 576])

    nc.compile()
    return nc


def kernel(**inputs):
    f32, f16 = np.float32, np.float16
    persp = np.asarray(inputs['perspective'], dtype=f32)
    t_w = np.asarray(inputs['t_w'], dtype=f32)
    z_w = np.asarray(inputs['z_w'], dtype=f32)
    if 'nc' not in _CACHED:
        _CACHED['nc'] = build()
    nc = _CACHED['nc']
    KT = [max(1, c // 128) for c in CR]

    # local query order: col L = 64*j + q  ->  global n = 36*q + 18*h + j
    Lq = np.arange(QL)
    qv, jv = Lq % 64, Lq // 64
    twt16 = np.zeros((128, 128), f16)
    twt = np.ascontiguousarray(t_w.T)
    twt16[:, 0:64] = twt[0:128].astype(f16)
    twt16[:, 64:128] = twt[128:256].astype(f16)
    zwt = np.ascontiguousarray(z_w.T)
    zw65 = np.zeros((128, 650), f32)
    for t in range(2):
        for si in range(5):
            zw65[:, 325 * t + 65 * si:325 * t + 65 * si + 64] = \
                z_w[128 * t:128 * t + 128, :]
    gmp = np.zeros((128, 13), f32)
    for t in range(2):
        for si in range(5):
            gmp[:, 5 * t + si] = np.asarray(
                inputs[f'bn{SCHED[si]}_g'], f32)[128 * t:128 * t + 128]
        gmp[:, 10 + t] = sum(np.asarray(inputs[f'bn{s}_b'], f32)
                             for s in range(5))[128 * t:128 * t + 128]
    gmp[:, 12] = EPS
    i128 = np.eye(128, dtype=f16)
    nkt = sum(KT)
    pwt = np.zeros((128, 64 * nkt), f16)
    gwt = np.zeros((128, 64 * nkt), f16)
    koff = 0
    for s in range(5):
        pw = np.asarray(inputs[f'p{s}_w'], f32).T
        gw = np.asarray(inputs[f'g{s}_w'], f32).T
        for kk in range(KT[s]):
            r0, r1 = 128 * kk, min(128 * kk + 128, CR[s])
            pwt[0:r1 - r0, koff:koff + 64] = pw[r0:r1].astype(f16)
            gwt[0:r1 - r0, koff:koff + 64] = gw[r0:r1].astype(f16)
            koff += 64

    in_maps = []
    for i in range(4):
        for h in range(2):
            nglob = 36 * qv + 18 * h + jv
            pi = persp[i].reshape(CP, 2304)[:, nglob]
            p16 = np.zeros((128, 2 * QL), f16)
            p16[:, 0:QL] = pi[0:128].astype(f16)
            p16[:, QL:] = pi[128:256].astype(f16)
            m = {"persp": p16, "twt": twt16,
                 "zwt16": np.concatenate([zwt, zwt], axis=0).astype(f16),
                 "zwtf": zwt, "zw65": zw65, "gmp": gmp, "i128": i128,
                 "pwt": pwt, "gwt": gwt}
            for s in range(5):
                rs = np.asarray(inputs[f'response{s}'], f32)[i].reshape(CR[s], MS[s])
                rt = np.zeros((min(CR[s], 128), KT[s] * MS[s]), f16)
                for kk in range(KT[s]):
                    r0, r1 = 128 * kk, min(128 * kk + 128, CR[s])
                    rt[0:r1 - r0, MS[s] * kk:MS[s] * kk + MS[s]] = \
                        rs[r0:r1].astype(f16)
                m[f"resp{s}"] = rt
            in_maps.append(m)
    res = bass_utils.run_bass_kernel_spmd(nc, in_maps,
                                          core_ids=list(range(NCORES)))
    _CACHED['res'] = res
    out = np.zeros((4, CP, 2304), np.float32)
    for i in range(4):
        for h in range(2):
            o = res.results[i * 2 + h]["out"]
            full = np.concatenate([o[:, 0:QL], o[:, QL:]], axis=0)
            out[i][:, QL * h:QL * h + QL] = full
    return out.reshape(4, CP, 48, 48)


if __name__ == "__main__":
    from concourse.timeline_sim import TimelineSim
    nc = build()
    tl = TimelineSim(nc, trace=False)
    print(f"TimelineSim: {tl.simulate():.0f} ns")


# revision 31
# speedup vs baseline: 1.0056x; 1.0056x over previous
"""Trainium2 Bass kernel for nn_CNL_5 (5-scale context non-local block).

Sharding: 8 cores = 4 samples x 2 query-subsets. Local query column order is
L = 64*j + q  (q = z-conv input channel = n//36-block, j = n%18), chosen so the
xbar DMA transpose (out[p,b,c] = in[c,128b+p]) directly yields the z-conv
operand x[q, pixel] with j-parity split across partition halves. outT is
padded to 640-col halves so each query-half transposes independently.

BN batch stats travel as per-channel (s1, s2) quadratic forms [128,20] through
one small AllGather; heavy math is fp16/bf16 on the PE at 1 cyc/row.
"""
import numpy as np
import ml_dtypes
from contextlib import ExitStack

import concourse.bass as bass
import concourse.bacc as bacc
import concourse.tile as tile
from concourse import mybir
from concourse import bass_utils
from concourse.alu_op_type import AluOpType

F32 = mybir.dt.float32
F16 = mybir.dt.float16
BF16 = mybir.dt.bfloat16
AFT = mybir.ActivationFunctionType
AXX = mybir.AxisListType.X

NCORES = 8
CP = 256
QL = 1152
CR = [64, 256, 512, 1024, 2048]
MS = [2304, 2304, 576, 144, 36]
CSH = [0.0, 10.0, 15.0, 25.0, 40.0]
EPS = 1e-5
NPIX = 4 * 2304.0
SCHED = [4, 2, 3, 1, 0]
DEBUG = False

_CACHED = {}


def mtiles(M):
    out, off = [], 0
    while off < M:
        w = min(128, M - off)
        out.append((off, w))
        off += w
    return out


def chunks512(N):
    out, off = [], 0
    while off < N:
        w = min(512, N - off)
        out.append((off, w))
        off += w
    return out


def build():
    nc = bacc.Bacc("TRN2", target_bir_lowering=False, debug=False,
                   num_devices=NCORES)
    KT = [len(mtiles(c)) for c in CR]
    persp_d = nc.dram_tensor("persp", [128, 2 * QL], F16, kind="ExternalInput").ap()
    twt_d = nc.dram_tensor("twt", [128, 128], F16, kind="ExternalInput").ap()
    resp_d = [nc.dram_tensor(f"resp{s}", [min(CR[s], 128), KT[s] * MS[s]], F16,
                             kind="ExternalInput").ap() for s in range(5)]
    pwt_d = nc.dram_tensor("pwt", [128, 64 * sum(KT)], F16, kind="ExternalInput").ap()
    gwt_d = nc.dram_tensor("gwt", [128, 64 * sum(KT)], F16, kind="ExternalInput").ap()
    zwt16_d = nc.dram_tensor("zwt16", [128, CP], F16, kind="ExternalInput").ap()
    zwtf_d = nc.dram_tensor("zwtf", [64, CP], F32, kind="ExternalInput").ap()
    zw65_d = nc.dram_tensor("zw65", [128, 650], F32, kind="ExternalInput").ap()
    gmp_d = nc.dram_tensor("gmp", [128, 13], F32, kind="ExternalInput").ap()
    i128_d = nc.dram_tensor("i128", [128, 128], F16, kind="ExternalInput").ap()
    out_d = nc.dram_tensor("out", [128, 2 * QL], F32, kind="ExternalOutput").ap()
    dbg = {}
    if DEBUG:
        for nm, shp, dt in [("d_t16", [64, QL], F16), ("d_p16", [64, MS[2]], F16),
                            ("d_outT", [64, 1280], F16), ("d_xw0", [128, 320], F16),
                            ("d_xw1", [128, 320], F16), ("d_Gcat", [64, 325], F32),
                            ("d_arin", [128, 20], F32), ("d_stats", [128, 20], F32),
                            ("d_a16", [128, 10], F16), ("d_W0", [128, 256], F16),
                            ("d_opn", [64, 576], F32), ("d_opd", [64, 576], F32),
                            ("d_rc", [64, 576], F32), ("d_et", [128, 576], F16),
                            ("d_gall", [128, 640], F32)]:
            dbg[nm] = nc.dram_tensor(nm, shp, dt, kind="ExternalOutput").ap()

    with tile.TileContext(nc) as tc, ExitStack() as ctx:
        sb = ctx.enter_context(tc.tile_pool(name="sb", bufs=1))
        p2 = ctx.enter_context(tc.tile_pool(name="p2", bufs=2))
        et3 = ctx.enter_context(tc.tile_pool(name="et3", bufs=3))
        p16p = ctx.enter_context(tc.tile_pool(name="p16p", bufs=3))
        dram = ctx.enter_context(tc.tile_pool(name="dram", bufs=1, space="DRAM"))
        psc = ctx.enter_context(tc.tile_pool(name="psc", bufs=2, space="PSUM"))
        pop = ctx.enter_context(tc.tile_pool(name="pop", bufs=1, space="PSUM"))
        pcv = ctx.enter_context(tc.tile_pool(name="pcv", bufs=1, space="PSUM"))
        pgx = ctx.enter_context(tc.tile_pool(name="pgx", bufs=1, space="PSUM"))

        # ---------------- static loads ----------------
        twt_sb = sb.tile([128, 128], F16, tag="twt", name="twt")
        nc.sync.dma_start(twt_sb[:], twt_d)
        persp_sb = sb.tile([128, 2 * QL], F16, tag="persp", name="persp")
        nc.sync.dma_start(persp_sb[:], persp_d)
        pwt_sb = sb.tile([128, 64 * sum(KT)], F16, tag="pwt", name="pwt")
        nc.sync.dma_start(pwt_sb[:], pwt_d)
        resp_sb = [None] * 5
        s0_ = SCHED[0]
        r = sb.tile([min(CR[s0_], 128), KT[s0_] * MS[s0_]], F16,
                    tag=f"resp{s0_}", name=f"resp{s0_}")
        nc.sync.dma_start(r[:], resp_d[s0_])
        resp_sb[s0_] = r
        gwt_sb = sb.tile([128, 64 * sum(KT)], F16, tag="gwt", name="gwt")
        nc.sync.dma_start(gwt_sb[:], gwt_d)
        for s in SCHED[1:]:
            r = sb.tile([min(CR[s], 128), KT[s] * MS[s]], F16,
                        tag=f"resp{s}", name=f"resp{s}")
            nc.sync.dma_start(r[:], resp_d[s])
            resp_sb[s] = r
        KOFF = [64 * sum(KT[:s]) for s in range(5)]
        zwt16_sb = sb.tile([128, CP], F16, tag="zwt16", name="zwt16")
        nc.sync.dma_start(zwt16_sb[:], zwt16_d)
        zwtf_sb = sb.tile([64, CP], F32, tag="zwtf", name="zwtf")
        nc.sync.dma_start(zwtf_sb[:], zwtf_d)
        zw65g_sb = sb.tile([128, 650], F32, tag="zw65", name="zw65")
        nc.sync.dma_start(zw65g_sb[:], zw65_d)
        gmp_sb = sb.tile([128, 13], F32, tag="gmp", name="gmp")
        nc.sync.dma_start(gmp_sb[:], gmp_d)
        i128_sb = sb.tile([128, 128], F16, tag="i128", name="i128")
        nc.sync.dma_start(i128_sb[:], i128_d)
        bias_sb = []
        for s in range(5):
            bt = sb.tile([128, 1], F32, tag=f"bias{s}", name=f"bias{s}")
            nc.vector.memset(bt[:], -CSH[s])
            bias_sb.append(bt)
        ones128 = sb.tile([1, 128], F16, tag="ones128", name="ones128")
        nc.vector.memset(ones128[:], 1.0)
        onesc = sb.tile([64, 1], F16, tag="onesc", name="onesc")
        nc.vector.memset(onesc[:], 1.0)
        g_all = [sb.tile([128, 128 * 18], BF16, tag=f"gall{i}", name=f"gall{i}")
                 for i in range(3)]
        for i in range(3):
            nc.gpsimd.memset(
                g_all[i][:].rearrange("p (k c) -> p k c", c=128)[:, :, 64:128], 1.0)
        # outT ring: pre-zero the 64-col pads of both ring slots
        oT = [p2.tile([64, 1280], F16, tag="outT", name=f"outTz{i}")
              for i in range(2)]
        for i in range(2):
            for h in range(2):
                nc.gpsimd.memset(oT[i][:, 640 * h + 576:640 * h + 640], 0.0)

        # ---------------- t conv: t16 [64, QL] ----------------
        t16 = sb.tile([64, QL], F16, tag="t16", name="t16")
        for off, w in chunks512(QL):
            tp = pgx.tile([128, 512], F32, tag="gx", name="gx")
            for kk in range(2):
                nc.tensor.matmul(tp[0:64, 0:w], twt_sb[:, 64 * kk:64 * kk + 64],
                                 persp_sb[:, QL * kk + off:QL * kk + off + w],
                                 start=(kk == 0), stop=(kk == 1))
            nc.vector.tensor_copy(t16[:, off:off + w], tp[0:64, 0:w])

        # ---------------- per-scale p/g convs (as deferrable units) --------
        p16_sb, xw_sb = {}, {}

        def conv_units(s, evac_eng):
            nct = mtiles(CR[s])
            p16 = p16p.tile([64, MS[s]], F16, tag="p16", name=f"p16_{s}")
            p16_sb[s] = p16
            units = []

            def p_unit(off, w):
                def emit():
                    pp = pcv.tile([128, 512], F32, tag="cv", name="cv")
                    for kk, (co, cw) in enumerate(nct):
                        nc.tensor.matmul(
                            pp[0:64, 0:w],
                            pwt_sb[0:cw, KOFF[s] + 64 * kk:KOFF[s] + 64 * kk + 64],
                            resp_sb[s][0:cw, MS[s] * kk + off:MS[s] * kk + off + w],
                            start=(kk == 0), stop=(kk == len(nct) - 1))
                    evac_eng.tensor_copy(p16[:, off:off + w], pp[0:64, 0:w])
                return emit

            def g_unit(b0, batch):
                def emit():
                    ga = g_all[SCHED.index(s) % 3]
                    gp = pcv.tile([128, 512], F32, tag="cv", name="cv")
                    for k, (moff, mw) in enumerate(batch):
                        for kk, (co, cw) in enumerate(nct):
                            nc.tensor.matmul(
                                gp[0:mw, 64 * k:64 * k + 64],
                                resp_sb[s][0:cw, MS[s] * kk + moff:MS[s] * kk + moff + mw],
                                gwt_sb[0:cw, KOFF[s] + 64 * kk:KOFF[s] + 64 * kk + 64],
                                start=(kk == 0), stop=(kk == len(nct) - 1))
                    dst = ga[:].rearrange("p (k c) -> p k c", c=128)[
                        :, b0:b0 + len(batch), 0:64]
                    src = gp[:].rearrange("p (k c) -> p k c", c=64)[
                        :, 0:len(batch), :]
                    evac_eng.tensor_copy(dst, src)
                return emit

            for off, w in chunks512(MS[s]):
                units.append((s, p_unit(off, w)))
            mts = mtiles(MS[s])
            for b0 in range(0, len(mts), 8):
                units.append((s, g_unit(b0, mts[b0:b0 + 8])))
            return units

        for _, u in conv_units(SCHED[0], nc.vector):
            u()
        pending = conv_units(SCHED[1], nc.vector)

        # ---------------- attention per scale ----------------
        arin_sb = sb.tile([128, 20], F32, tag="arin", name="arin")
        G_cat = sb.tile([64, 325], F32, tag="G_cat", name="G_cat")
        for si, s in enumerate(SCHED):
            mts = mtiles(MS[s])
            ga = g_all[si % 3]
            while pending and pending[0][0] == s:
                pending.pop(0)[1]()
            if si + 2 < 5:
                pending += conv_units(SCHED[si + 2], nc.vector)
            outT = p2.tile([64, 1280], F16, tag="outT", name=f"outT{s}")
            gm = pgx.tile([128, 512], F32, tag="gx", name="gx")
            for h in range(2):
                op = pop.tile([128, 576], F32, tag="op", name="op")
                for k, (moff, mw) in enumerate(mts):
                    sc = psc.tile([128, 576], F32, tag="sc", name="sc")
                    for co, cw in ((0, 512), (512, 64)):
                        nc.tensor.matmul(
                            sc[0:mw, co:co + cw],
                            p16_sb[s][:, moff:moff + mw],
                            t16[:, 576 * h + co:576 * h + co + cw],
                            start=True, stop=True)
                    et = et3.tile([128, 576], BF16, tag="et", name="et")
                    nc.scalar.activation(et[0:mw, :], sc[0:mw, :], AFT.Exp,
                                         bias=bias_sb[s][0:mw, :])
                    if DEBUG and si == 0 and h == 0 and k == 0:
                        de = sb.tile([128, 576], F16, tag="dbg3", name="dbg3")
                        nc.vector.tensor_copy(de[:], et[:])
                        nc.sync.dma_start(dbg["d_et"], de[:])
                    for co, cw in ((0, 512), (512, 64)):
                        nc.tensor.matmul(
                            op[:, co:co + cw],
                            ga[0:mw, 128 * k:128 * k + 128],
                            et[0:mw, co:co + cw],
                            start=(k == 0), stop=(k == len(mts) - 1))
                    if pending and (k % 2 == 1 or len(mts) < 10):
                        pending.pop(0)[1]()
                rc = p2.tile([64, 576], F32, tag="rc", name="rc")
                if DEBUG and si == 0 and h == 1:
                    dn = sb.tile([64, 576], F32, tag="dbg1", name="dbg1")
                    nc.vector.tensor_copy(dn[:], op[0:64, :])
                    nc.sync.dma_start(dbg["d_opn"], dn[:])
                    dd = sb.tile([64, 576], F32, tag="dbg2", name="dbg2")
                    nc.vector.tensor_copy(dd[:], op[64:128, :])
                    nc.sync.dma_start(dbg["d_opd"], dd[:])
                nc.vector.reciprocal(rc[:], op[64:128, :])
                if DEBUG and si == 0 and h == 1:
                    nc.sync.dma_start(dbg["d_rc"], rc[:])
                nc.vector.tensor_tensor(outT[:, 640 * h:640 * h + 576],
                                        op[0:64, :], rc[:], op=AluOpType.mult)
                # per-half xbar transpose + svec + gram
                xw = sb.tile([128, 5 * 64], F16, tag=f"xw{s}_{h}",
                             name=f"xw{s}_{h}")
                xw_sb[(s, h)] = xw
                nc.sync.dma_start_transpose(
                    xw[:].rearrange("p (b c) -> p b c", c=64),
                    outT[:, 640 * h:640 * h + 640])
                for j in range(9):
                    nc.tensor.matmul(
                        gm[0:64, 0:64],
                        outT[:, 640 * h + 64 * j:640 * h + 64 * j + 64],
                        outT[:, 640 * h + 64 * j:640 * h + 64 * j + 64],
                        start=(h == 0 and j == 0), stop=(h == 1 and j == 8))
            if DEBUG and si == 0:
                nc.sync.dma_start(dbg["d_outT"], outT[:])
            # order in one bank: gram -> G-copy -> svec-sum -> svrow-copy ->
            # transpose -> G-col copy (chained via overlapping regions, since
            # a matmul 'start' resets the whole bank)
            nc.vector.tensor_copy(G_cat[:, 65 * si:65 * si + 64],
                                  gm[0:64, 0:64])
            for h in range(2):
                for j in range(9):
                    nc.tensor.matmul(
                        gm[0:1, 0:64], onesc[:],
                        outT[:, 640 * h + 64 * j:640 * h + 64 * j + 64],
                        start=(h == 0 and j == 0), stop=(h == 1 and j == 8))
            svrow = p2.tile([1, 64], F16, tag="svrow", name="svrow")
            nc.vector.tensor_copy(svrow[:], gm[0:1, 0:64])
            nc.tensor.matmul(gm[0:64, 0:1], svrow[:], onesc[0:1, 0:1],
                             start=True, stop=True)
            nc.vector.tensor_copy(G_cat[:, 65 * si + 64:65 * si + 65],
                                  gm[0:64, 0:1])
            if si == 3:
                # BN partials for first 4 scheduled scales (off critical path)
                for t in range(2):
                    zgp = (pgx if t == 0 else pcv).tile(
                        [128, 512], F32, tag="gx" if t == 0 else "cv",
                        name="zgP")
                    nc.tensor.matmul(zgp[:, 0:260],
                                     zwtf_sb[:, 128 * t:128 * t + 128],
                                     G_cat[:, 0:260], start=True, stop=True)
                    zzp = p2.tile([128, 260], F32, tag="zz", name="zz")
                    nc.vector.tensor_tensor(
                        zzp[:], zgp[:, 0:260],
                        zw65g_sb[:, 325 * t:325 * t + 260],
                        op=AluOpType.mult)
                    nc.vector.tensor_reduce(
                        arin_sb[:, 10 * t:10 * t + 4],
                        zzp[:].rearrange("p (s c) -> p s c", c=65), AXX,
                        AluOpType.add)
                    nc.vector.tensor_copy(
                        arin_sb[:, 10 * t + 5:10 * t + 9],
                        bass.AP(tensor=zgp[:].tensor,
                                offset=zgp[:].offset + 64,
                                ap=[[zgp[:].ap[0][0], 128], [65, 4]]))

        if DEBUG:
            dg = sb.tile([128, 640], F32, tag="dbg4", name="dbg4")
            nc.vector.tensor_copy(dg[:], g_all[0][:, 0:640])
            nc.sync.dma_start(dbg["d_gall"], dg[:])
            nc.sync.dma_start(dbg["d_t16"], t16[:])
            nc.sync.dma_start(dbg["d_p16"], p16_sb[2][:])
            nc.sync.dma_start(dbg["d_Gcat"], G_cat[:])
            nc.sync.dma_start(dbg["d_xw0"], xw_sb[(2, 0)][:])
            nc.sync.dma_start(dbg["d_xw1"], xw_sb[(2, 1)][:])
        # last scheduled scale's BN stats (cols 260:325 of G_cat)
        for t in range(2):
            zg = (pgx if t == 0 else pcv).tile(
                [128, 512], F32, tag="gx" if t == 0 else "cv", name="zgL")
            nc.tensor.matmul(zg[:, 0:65], zwtf_sb[:, 128 * t:128 * t + 128],
                             G_cat[:, 260:325], start=True, stop=True)
            zzl = p2.tile([128, 65], F32, tag="zzL", name="zzL")
            nc.vector.tensor_tensor(zzl[:], zg[:, 0:65],
                                    zw65g_sb[:, 325 * t + 260:325 * t + 325],
                                    op=AluOpType.mult)
            nc.vector.tensor_reduce(arin_sb[:, 10 * t + 4:10 * t + 5],
                                    zzl[:], AXX, AluOpType.add)
            nc.vector.tensor_copy(arin_sb[:, 10 * t + 9:10 * t + 10],
                                  zg[:, 64:65])

        if DEBUG:
            nc.sync.dma_start(dbg["d_arin"], arin_sb[:])
        # ---------------- stats AllGather ----------------
        arin = dram.tile([128, 20], F32, name="arin_d")
        arout = dram.tile([128 * NCORES, 20], F32, name="arout_d")
        nc.sync.dma_start(arin[:], arin_sb[:])
        nc.gpsimd.collective_compute(
            "AllGather", AluOpType.bypass,
            replica_groups=[list(range(NCORES))],
            ins=[arin.opt()], outs=[arout.opt()])
        gath = sb.tile([128, 160], F32, tag="gath", name="gath")
        src = bass.AP(tensor=arout[:].tensor, offset=arout[:].offset,
                      ap=[[20, 128], [2560, 8], [1, 20]])
        nc.sync.dma_start(gath[:], src)
        stats = sb.tile([128, 20], F32, tag="stats", name="stats")
        nc.vector.tensor_reduce(
            stats[:],
            bass.AP(tensor=gath[:].tensor, offset=gath[:].offset,
                    ap=[[gath[:].ap[0][0], 128], [1, 20], [20, 8]]),
            AXX, AluOpType.add)

        if DEBUG:
            nc.sync.dma_start(dbg["d_stats"], stats[:])
        # ---------------- BN coefficients (SCHED order) ----------------
        a16 = sb.tile([128, 10], F16, tag="a16", name="a16")
        bacc_t = [sb.tile([128, 1], F32, tag=f"bacc{t}", name=f"bacc{t}")
                  for t in range(2)]
        for t in range(2):
            s2v = stats[:, 10 * t:10 * t + 5]
            s1v = stats[:, 10 * t + 5:10 * t + 10]
            mean = p2.tile([128, 5], F32, tag="mean", name="mean")
            nc.vector.tensor_scalar_mul(mean[:], s1v, 1.0 / NPIX)
            m2 = p2.tile([128, 5], F32, tag="m2", name="m2")
            nc.vector.tensor_tensor(m2[:], mean[:], mean[:], op=AluOpType.mult)
            var = p2.tile([128, 5], F32, tag="var", name="var")
            nc.vector.scalar_tensor_tensor(var[:], s2v, 1.0 / NPIX, m2[:],
                                           op0=AluOpType.mult,
                                           op1=AluOpType.subtract)
            sq = p2.tile([128, 5], F32, tag="sq", name="sq")
            nc.scalar.activation(sq[:], var[:], AFT.Sqrt,
                                 bias=gmp_sb[:, 12:13])
            rinv = p2.tile([128, 5], F32, tag="rinv", name="rinv")
            nc.vector.reciprocal_approx_fast(rinv[:], sq[:])
            af = p2.tile([128, 5], F32, tag="af", name="af")
            nc.vector.tensor_tensor(af[:], rinv[:], gmp_sb[:, 5 * t:5 * t + 5],
                                    op=AluOpType.mult)
            nc.vector.tensor_copy(a16[:, 5 * t:5 * t + 5], af[:])
            tmb = p2.tile([128, 5], F32, tag="tmb", name="tmb")
            nc.vector.tensor_tensor(tmb[:], af[:], mean[:], op=AluOpType.mult)
            tmbr = p2.tile([128, 1], F32, tag="tmbr", name="tmbr")
            nc.vector.tensor_reduce(tmbr[:], tmb[:], AXX, AluOpType.add)
            nc.vector.tensor_tensor(bacc_t[t][:], gmp_sb[:, 10 + t:11 + t],
                                    tmbr[:], op=AluOpType.subtract)
        # a5cat rows via PE transposes: one accumulation group per bank
        # (start zeroes the bank; disjoint-column matmuls add into zeros),
        # then 3 wide copies instead of 10 narrow ones
        a5cat = sb.tile([1, 1280], F16, tag="a5cat", name="a5cat")
        banks = [(pgx, "gx", 0, 4), (pcv, "cv", 4, 8), (psc, "sc", 8, 10)]
        for pool, tag, i0, i1 in banks:
            atp = pool.tile([128, 512], F32, tag=tag, name="tp")
            for ii in range(i0, i1):
                si, t = ii // 2, ii % 2
                nc.tensor.matmul(
                    atp[0:1, 128 * (ii - i0):128 * (ii - i0) + 128],
                    a16[:, 5 * t + si:5 * t + si + 1],
                    i128_sb[:], start=(ii == i0), stop=(ii == i1 - 1))
            nc.vector.tensor_copy(a5cat[0:1, 128 * i0:128 * i1],
                                  atp[0:1, 0:128 * (i1 - i0)])
        W_sb = []
        for si in range(5):
            abp = (pcv if si % 2 else pgx).tile(
                [128, 512], F32, tag="cv" if si % 2 else "gx", name="ab")
            nc.tensor.matmul(abp[:, 0:256], ones128[:],
                             a5cat[0:1, 256 * si:256 * si + 256],
                             start=True, stop=True)
            W = sb.tile([128, 256], F16, tag=f"W{si}", name=f"W{si}")
            nc.vector.tensor_tensor(W[:], zwt16_sb[:], abp[:, 0:256],
                                    op=AluOpType.mult)
            W_sb.append(W)

        if DEBUG:
            nc.sync.dma_start(dbg["d_a16"], a16[:])
            nc.sync.dma_start(dbg["d_W0"], W_sb[0][:])
        # ---------------- final matmul + store ----------------
        for t in range(2):
            out_sb = sb.tile([128, QL], F32, tag=f"osb{t}", name=f"osb{t}")
            for h in range(2):
                for par in range(2):
                    nb = 5 if par == 0 else 4
                    fp = psc.tile([128, 576], F32, tag="sc", name="sc")
                    for si in range(5):
                        nc.tensor.matmul(
                            fp[:, 0:64 * nb],
                            W_sb[si][64 * par:64 * par + 64,
                                     128 * t:128 * t + 128],
                            xw_sb[(SCHED[si], h)][64 * par:64 * par + 64,
                                                  0:64 * nb],
                            start=(si == 0), stop=(si == 4))
                    dst = bass.AP(
                        tensor=out_sb[:].tensor,
                        offset=out_sb[:].offset + 64 * (9 * h + par),
                        ap=[[out_sb[:].ap[0][0], 128], [128, nb], [1, 64]])
                    nc.vector.tensor_scalar_add(
                        dst,
                        fp[:].rearrange("p (b c) -> p b c", c=64)[:, 0:nb, :],
                        bacc_t[t][:])
                nc.sync.dma_start(
                    out_d[:, QL * t + 576 * h:QL * t + 576 * h + 576],
                    out_sb[:, 576 * h:576 * h + 576])

    nc.compile()
    return nc


def kernel(**inputs):
    f32, f16 = np.float32, np.float16
    persp = np.asarray(inputs['perspective'], dtype=f32)
    t_w = np.asarray(inputs['t_w'], dtype=f32)
    z_w = np.asarray(inputs['z_w'], dtype=f32)
    if 'nc' not in _CACHED:
        _CACHED['nc'] = build()
    nc = _CACHED['nc']
    KT = [max(1, c // 128) for c in CR]

    # local query order: col L = 64*j + q  ->  global n = 36*q + 18*h + j
    Lq = np.arange(QL)
    qv, jv = Lq % 64, Lq // 64
    twt16 = np.zeros((128, 128), f16)
    twt = np.ascontiguousarray(t_w.T)
    twt16[:, 0:64] = twt[0:128].astype(f16)
    twt16[:, 64:128] = twt[128:256].astype(f16)
    zwt = np.ascontiguousarray(z_w.T)
    zw65 = np.zeros((128, 650), f32)
    for t in range(2):
        for si in range(5):
            zw65[:, 325 * t + 65 * si:325 * t + 65 * si + 64] = \
                z_w[128 * t:128 * t + 128, :]
    gmp = np.zeros((128, 13), f32)
    for t in range(2):
        for si in range(5):
            gmp[:, 5 * t + si] = np.asarray(
                inputs[f'bn{SCHED[si]}_g'], f32)[128 * t:128 * t + 128]
        gmp[:, 10 + t] = sum(np.asarray(inputs[f'bn{s}_b'], f32)
                             for s in range(5))[128 * t:128 * t + 128]
    gmp[:, 12] = EPS
    i128 = np.eye(128, dtype=f16)
    nkt = sum(KT)
    pwt = np.zeros((128, 64 * nkt), f16)
    gwt = np.zeros((128, 64 * nkt), f16)
    koff = 0
    for s in range(5):
        pw = np.asarray(inputs[f'p{s}_w'], f32).T
        gw = np.asarray(inputs[f'g{s}_w'], f32).T
        for kk in range(KT[s]):
            r0, r1 = 128 * kk, min(128 * kk + 128, CR[s])
            pwt[0:r1 - r0, koff:koff + 64] = pw[r0:r1].astype(f16)
            gwt[0:r1 - r0, koff:koff + 64] = gw[r0:r1].astype(f16)
            koff += 64

    in_maps = []
    for i in range(4):
        for h in range(2):
            nglob = 36 * qv + 18 * h + jv
            pi = persp[i].reshape(CP, 2304)[:, nglob]
            p16 = np.zeros((128, 2 * QL), f16)
            p16[:, 0:QL] = pi[0:128].astype(f16)
            p16[:, QL:] = pi[128:256].astype(f16)
            m = {"persp": p16, "twt": twt16,
                 "zwt16": np.concatenate([zwt, zwt], axis=0).astype(f16),
                 "zwtf": zwt, "zw65": zw65, "gmp": gmp, "i128": i128,
                 "pwt": pwt, "gwt": gwt}
            for s in range(5):
                rs = np.asarray(inputs[f'response{s}'], f32)[i].reshape(CR[s], MS[s])
                rt = np.zeros((min(CR[s], 128), KT[s] * MS[s]), f16)
                for kk in range(KT[s]):
                    r0, r1 = 128 * kk, min(128 * kk + 128, CR[s])
                    rt[0:r1 - r0, MS[s] * kk:MS[s] * kk + MS[s]] = \
                        rs[r0:r1].astype(f16)
                m[f"resp{s}"] = rt
            in_maps.append(m)
    res = bass_utils.run_bass_kernel_spmd(nc, in_maps,
                                          core_ids=list(range(NCORES)))
    _CACHED['res'] = res
    out = np.zeros((4, CP, 2304), np.float32)
    for i in range(4):
        for h in range(2):
            o = res.results[i * 2 + h]["out"]
            full = np.concatenate([o[:, 0:QL], o[:, QL:]], axis=0)
            out[i][:, QL * h:QL * h + QL] = full
    return out.reshape(4, CP, 48, 48)


if __name__ == "__main__":
    from concourse.timeline_sim import TimelineSim
    nc = build()
    tl = TimelineSim(nc, trace=False)
    print(f"TimelineSim: {tl.simulate():.0f} ns")


# revision 32
# speedup vs baseline: 1.0105x; 1.0049x over previous
"""Trainium2 Bass kernel for nn_CNL_5 (5-scale context non-local block).

Sharding: 8 cores = 4 samples x 2 query-subsets. Local query column order is
L = 64*j + q  (q = z-conv input channel = n//36-block, j = n%18), chosen so the
xbar DMA transpose (out[p,b,c] = in[c,128b+p]) directly yields the z-conv
operand x[q, pixel] with j-parity split across partition halves. outT is
padded to 640-col halves so each query-half transposes independently.

BN batch stats travel as per-channel (s1, s2) quadratic forms [128,20] through
one small AllGather; heavy math is fp16/bf16 on the PE at 1 cyc/row.
"""
import numpy as np
import ml_dtypes
from contextlib import ExitStack

import concourse.bass as bass
import concourse.bacc as bacc
import concourse.tile as tile
from concourse import mybir
from concourse import bass_utils
from concourse.alu_op_type import AluOpType

F32 = mybir.dt.float32
F16 = mybir.dt.float16
BF16 = mybir.dt.bfloat16
AFT = mybir.ActivationFunctionType
AXX = mybir.AxisListType.X

NCORES = 8
CP = 256
QL = 1152
CR = [64, 256, 512, 1024, 2048]
MS = [2304, 2304, 576, 144, 36]
CSH = [0.0, 10.0, 15.0, 25.0, 40.0]
EPS = 1e-5
NPIX = 4 * 2304.0
SCHED = [4, 2, 3, 1, 0]
DEBUG = False

_CACHED = {}


def mtiles(M):
    out, off = [], 0
    while off < M:
        w = min(128, M - off)
        out.append((off, w))
        off += w
    return out


def chunks512(N):
    out, off = [], 0
    while off < N:
        w = min(512, N - off)
        out.append((off, w))
        off += w
    return out


def build():
    nc = bacc.Bacc("TRN2", target_bir_lowering=False, debug=False,
                   num_devices=NCORES)
    KT = [len(mtiles(c)) for c in CR]
    persp_d = nc.dram_tensor("persp", [128, 2 * QL], F16, kind="ExternalInput").ap()
    twt_d = nc.dram_tensor("twt", [128, 128], F16, kind="ExternalInput").ap()
    resp_d = [nc.dram_tensor(f"resp{s}", [min(CR[s], 128), KT[s] * MS[s]], F16,
                             kind="ExternalInput").ap() for s in range(5)]
    pwt_d = nc.dram_tensor("pwt", [128, 64 * sum(KT)], F16, kind="ExternalInput").ap()
    gwt_d = nc.dram_tensor("gwt", [128, 64 * sum(KT)], F16, kind="ExternalInput").ap()
    zwt16_d = nc.dram_tensor("zwt16", [128, CP], F16, kind="ExternalInput").ap()
    zwtf_d = nc.dram_tensor("zwtf", [64, CP], F32, kind="ExternalInput").ap()
    zw65_d = nc.dram_tensor("zw65", [128, 650], F32, kind="ExternalInput").ap()
    gmp_d = nc.dram_tensor("gmp", [128, 13], F32, kind="ExternalInput").ap()
    i128_d = nc.dram_tensor("i128", [128, 128], F16, kind="ExternalInput").ap()
    out_d = nc.dram_tensor("out", [128, 2 * QL], F32, kind="ExternalOutput").ap()
    dbg = {}
    if DEBUG:
        for nm, shp, dt in [("d_t16", [64, QL], F16), ("d_p16", [64, MS[2]], F16),
                            ("d_outT", [64, 1280], F16), ("d_xw0", [128, 320], F16),
                            ("d_xw1", [128, 320], F16), ("d_Gcat", [64, 325], F32),
                            ("d_arin", [128, 20], F32), ("d_stats", [128, 20], F32),
                            ("d_a16", [128, 10], F16), ("d_W0", [128, 256], F16),
                            ("d_opn", [64, 576], F32), ("d_opd", [64, 576], F32),
                            ("d_rc", [64, 576], F32), ("d_et", [128, 576], F16),
                            ("d_gall", [128, 640], F32)]:
            dbg[nm] = nc.dram_tensor(nm, shp, dt, kind="ExternalOutput").ap()

    with tile.TileContext(nc) as tc, ExitStack() as ctx:
        sb = ctx.enter_context(tc.tile_pool(name="sb", bufs=1))
        p2 = ctx.enter_context(tc.tile_pool(name="p2", bufs=2))
        et3 = ctx.enter_context(tc.tile_pool(name="et3", bufs=3))
        p16p = ctx.enter_context(tc.tile_pool(name="p16p", bufs=3))
        dram = ctx.enter_context(tc.tile_pool(name="dram", bufs=1, space="DRAM"))
        psc = ctx.enter_context(tc.tile_pool(name="psc", bufs=2, space="PSUM"))
        pop = ctx.enter_context(tc.tile_pool(name="pop", bufs=1, space="PSUM"))
        pcv = ctx.enter_context(tc.tile_pool(name="pcv", bufs=1, space="PSUM"))
        pgx = ctx.enter_context(tc.tile_pool(name="pgx", bufs=1, space="PSUM"))

        # ---------------- static loads ----------------
        twt_sb = sb.tile([128, 128], F16, tag="twt", name="twt")
        nc.sync.dma_start(twt_sb[:], twt_d)
        persp_sb = sb.tile([128, 2 * QL], F16, tag="persp", name="persp")
        nc.sync.dma_start(persp_sb[:], persp_d)
        pwt_sb = sb.tile([128, 64 * sum(KT)], F16, tag="pwt", name="pwt")
        nc.sync.dma_start(pwt_sb[:], pwt_d)
        resp_sb = [None] * 5
        s0_ = SCHED[0]
        r = sb.tile([min(CR[s0_], 128), KT[s0_] * MS[s0_]], F16,
                    tag=f"resp{s0_}", name=f"resp{s0_}")
        nc.sync.dma_start(r[:], resp_d[s0_])
        resp_sb[s0_] = r
        gwt_sb = sb.tile([128, 64 * sum(KT)], F16, tag="gwt", name="gwt")
        nc.sync.dma_start(gwt_sb[:], gwt_d)
        for s in SCHED[1:]:
            r = sb.tile([min(CR[s], 128), KT[s] * MS[s]], F16,
                        tag=f"resp{s}", name=f"resp{s}")
            nc.sync.dma_start(r[:], resp_d[s])
            resp_sb[s] = r
        KOFF = [64 * sum(KT[:s]) for s in range(5)]
        zwt16_sb = sb.tile([128, CP], F16, tag="zwt16", name="zwt16")
        nc.sync.dma_start(zwt16_sb[:], zwt16_d)
        zwtf_sb = sb.tile([64, CP], F32, tag="zwtf", name="zwtf")
        nc.sync.dma_start(zwtf_sb[:], zwtf_d)
        zw65g_sb = sb.tile([128, 650], F32, tag="zw65", name="zw65")
        nc.sync.dma_start(zw65g_sb[:], zw65_d)
        gmp_sb = sb.tile([128, 13], F32, tag="gmp", name="gmp")
        nc.sync.dma_start(gmp_sb[:], gmp_d)
        i128_sb = sb.tile([128, 128], F16, tag="i128", name="i128")
        nc.sync.dma_start(i128_sb[:], i128_d)
        bias_sb = []
        for s in range(5):
            bt = sb.tile([128, 1], F32, tag=f"bias{s}", name=f"bias{s}")
            nc.vector.memset(bt[:], -CSH[s])
            bias_sb.append(bt)
        ones128 = sb.tile([1, 128], F16, tag="ones128", name="ones128")
        nc.vector.memset(ones128[:], 1.0)
        onesc = sb.tile([64, 1], F16, tag="onesc", name="onesc")
        nc.vector.memset(onesc[:], 1.0)
        g_all = [sb.tile([128, 128 * 18], BF16, tag=f"gall{i}", name=f"gall{i}")
                 for i in range(3)]
        for i in range(3):
            nc.gpsimd.memset(
                g_all[i][:].rearrange("p (k c) -> p k c", c=128)[:, :, 64:128], 1.0)
        # outT ring: pre-zero the 64-col pads of both ring slots
        oT = [p2.tile([64, 1280], F16, tag="outT", name=f"outTz{i}")
              for i in range(2)]
        for i in range(2):
            for h in range(2):
                nc.gpsimd.memset(oT[i][:, 640 * h + 576:640 * h + 640], 0.0)

        # ---------------- t conv: t16 [64, QL] ----------------
        t16 = sb.tile([64, QL], F16, tag="t16", name="t16")
        for off, w in chunks512(QL):
            tp = pgx.tile([128, 512], F32, tag="gx", name="gx")
            for kk in range(2):
                nc.tensor.matmul(tp[0:64, 0:w], twt_sb[:, 64 * kk:64 * kk + 64],
                                 persp_sb[:, QL * kk + off:QL * kk + off + w],
                                 start=(kk == 0), stop=(kk == 1))
            nc.vector.tensor_copy(t16[:, off:off + w], tp[0:64, 0:w])

        # ---------------- per-scale p/g convs (as deferrable units) --------
        p16_sb, xw_sb = {}, {}

        def conv_units(s, evac_eng):
            nct = mtiles(CR[s])
            p16 = p16p.tile([64, MS[s]], F16, tag="p16", name=f"p16_{s}")
            p16_sb[s] = p16
            units = []

            def p_unit(off, w):
                def emit():
                    pp = pcv.tile([128, 512], F32, tag="cv", name="cv")
                    for kk, (co, cw) in enumerate(nct):
                        nc.tensor.matmul(
                            pp[0:64, 0:w],
                            pwt_sb[0:cw, KOFF[s] + 64 * kk:KOFF[s] + 64 * kk + 64],
                            resp_sb[s][0:cw, MS[s] * kk + off:MS[s] * kk + off + w],
                            start=(kk == 0), stop=(kk == len(nct) - 1))
                    evac_eng.tensor_copy(p16[:, off:off + w], pp[0:64, 0:w])
                return emit

            def g_unit(b0, batch):
                def emit():
                    ga = g_all[SCHED.index(s) % 3]
                    gp = pcv.tile([128, 512], F32, tag="cv", name="cv")
                    for k, (moff, mw) in enumerate(batch):
                        for kk, (co, cw) in enumerate(nct):
                            nc.tensor.matmul(
                                gp[0:mw, 64 * k:64 * k + 64],
                                resp_sb[s][0:cw, MS[s] * kk + moff:MS[s] * kk + moff + mw],
                                gwt_sb[0:cw, KOFF[s] + 64 * kk:KOFF[s] + 64 * kk + 64],
                                start=(kk == 0), stop=(kk == len(nct) - 1))
                    dst = ga[:].rearrange("p (k c) -> p k c", c=128)[
                        :, b0:b0 + len(batch), 0:64]
                    src = gp[:].rearrange("p (k c) -> p k c", c=64)[
                        :, 0:len(batch), :]
                    evac_eng.tensor_copy(dst, src)
                return emit

            for off, w in chunks512(MS[s]):
                units.append((s, p_unit(off, w)))
            mts = mtiles(MS[s])
            for b0 in range(0, len(mts), 8):
                units.append((s, g_unit(b0, mts[b0:b0 + 8])))
            return units

        for _, u in conv_units(SCHED[0], nc.vector):
            u()
        pending = conv_units(SCHED[1], nc.vector)

        # ---------------- attention per scale ----------------
        arin_sb = sb.tile([128, 20], F32, tag="arin", name="arin")
        G_cat = sb.tile([64, 325], F32, tag="G_cat", name="G_cat")
        for si, s in enumerate(SCHED):
            mts = mtiles(MS[s])
            ga = g_all[si % 3]
            while pending and pending[0][0] == s:
                pending.pop(0)[1]()
            if si + 2 < 5:
                pending += conv_units(SCHED[si + 2], nc.vector)
            outT = p2.tile([64, 1280], F16, tag="outT", name=f"outT{s}")
            gm = pgx.tile([128, 512], F32, tag="gx", name="gx")
            for h in range(2):
                op = pop.tile([128, 576], F32, tag="op", name="op")
                for k, (moff, mw) in enumerate(mts):
                    sc = psc.tile([128, 576], F32, tag="sc", name="sc")
                    for co, cw in ((0, 512), (512, 64)):
                        nc.tensor.matmul(
                            sc[0:mw, co:co + cw],
                            p16_sb[s][:, moff:moff + mw],
                            t16[:, 576 * h + co:576 * h + co + cw],
                            start=True, stop=True)
                    et = et3.tile([128, 576], BF16, tag="et", name="et")
                    nc.scalar.activation(et[0:mw, :], sc[0:mw, :], AFT.Exp,
                                         bias=bias_sb[s][0:mw, :])
                    if DEBUG and si == 0 and h == 0 and k == 0:
                        de = sb.tile([128, 576], F16, tag="dbg3", name="dbg3")
                        nc.vector.tensor_copy(de[:], et[:])
                        nc.sync.dma_start(dbg["d_et"], de[:])
                    for co, cw in ((0, 512), (512, 64)):
                        nc.tensor.matmul(
                            op[:, co:co + cw],
                            ga[0:mw, 128 * k:128 * k + 128],
                            et[0:mw, co:co + cw],
                            start=(k == 0), stop=(k == len(mts) - 1))
                    if pending and (k % 2 == 1 or len(mts) < 10):
                        pending.pop(0)[1]()
                rc = p2.tile([64, 576], F32, tag="rc", name="rc")
                if DEBUG and si == 0 and h == 1:
                    dn = sb.tile([64, 576], F32, tag="dbg1", name="dbg1")
                    nc.vector.tensor_copy(dn[:], op[0:64, :])
                    nc.sync.dma_start(dbg["d_opn"], dn[:])
                    dd = sb.tile([64, 576], F32, tag="dbg2", name="dbg2")
                    nc.vector.tensor_copy(dd[:], op[64:128, :])
                    nc.sync.dma_start(dbg["d_opd"], dd[:])
                nc.vector.reciprocal(rc[:], op[64:128, :])
                if DEBUG and si == 0 and h == 1:
                    nc.sync.dma_start(dbg["d_rc"], rc[:])
                if len(mts) < 10:
                    # small scales: Act idles at the boundary — copy the
                    # numerator out on Act in parallel with the reciprocal so
                    # the op PSUM ring frees ~2x faster for the next half
                    num = p2.tile([64, 576], F32, tag="num", name="num")
                    nc.scalar.activation(num[:], op[0:64, :], AFT.Copy)
                    nc.vector.tensor_tensor(outT[:, 640 * h:640 * h + 576],
                                            num[:], rc[:], op=AluOpType.mult)
                else:
                    nc.vector.tensor_tensor(outT[:, 640 * h:640 * h + 576],
                                            op[0:64, :], rc[:],
                                            op=AluOpType.mult)
                # per-half xbar transpose + svec + gram
                xw = sb.tile([128, 5 * 64], F16, tag=f"xw{s}_{h}",
                             name=f"xw{s}_{h}")
                xw_sb[(s, h)] = xw
                nc.sync.dma_start_transpose(
                    xw[:].rearrange("p (b c) -> p b c", c=64),
                    outT[:, 640 * h:640 * h + 640])
                for j in range(9):
                    nc.tensor.matmul(
                        gm[0:64, 0:64],
                        outT[:, 640 * h + 64 * j:640 * h + 64 * j + 64],
                        outT[:, 640 * h + 64 * j:640 * h + 64 * j + 64],
                        start=(h == 0 and j == 0), stop=(h == 1 and j == 8))
            if DEBUG and si == 0:
                nc.sync.dma_start(dbg["d_outT"], outT[:])
            # order in one bank: gram -> G-copy -> svec-sum -> svrow-copy ->
            # transpose -> G-col copy (chained via overlapping regions, since
            # a matmul 'start' resets the whole bank)
            nc.vector.tensor_copy(G_cat[:, 65 * si:65 * si + 64],
                                  gm[0:64, 0:64])
            for h in range(2):
                for j in range(9):
                    nc.tensor.matmul(
                        gm[0:1, 0:64], onesc[:],
                        outT[:, 640 * h + 64 * j:640 * h + 64 * j + 64],
                        start=(h == 0 and j == 0), stop=(h == 1 and j == 8))
            svrow = p2.tile([1, 64], F16, tag="svrow", name="svrow")
            nc.vector.tensor_copy(svrow[:], gm[0:1, 0:64])
            nc.tensor.matmul(gm[0:64, 0:1], svrow[:], onesc[0:1, 0:1],
                             start=True, stop=True)
            nc.vector.tensor_copy(G_cat[:, 65 * si + 64:65 * si + 65],
                                  gm[0:64, 0:1])
            if si == 3:
                # BN partials for first 4 scheduled scales (off critical path)
                for t in range(2):
                    zgp = (pgx if t == 0 else pcv).tile(
                        [128, 512], F32, tag="gx" if t == 0 else "cv",
                        name="zgP")
                    nc.tensor.matmul(zgp[:, 0:260],
                                     zwtf_sb[:, 128 * t:128 * t + 128],
                                     G_cat[:, 0:260], start=True, stop=True)
                    zzp = p2.tile([128, 260], F32, tag="zz", name="zz")
                    nc.vector.tensor_tensor(
                        zzp[:], zgp[:, 0:260],
                        zw65g_sb[:, 325 * t:325 * t + 260],
                        op=AluOpType.mult)
                    nc.vector.tensor_reduce(
                        arin_sb[:, 10 * t:10 * t + 4],
                        zzp[:].rearrange("p (s c) -> p s c", c=65), AXX,
                        AluOpType.add)
                    nc.vector.tensor_copy(
                        arin_sb[:, 10 * t + 5:10 * t + 9],
                        bass.AP(tensor=zgp[:].tensor,
                                offset=zgp[:].offset + 64,
                                ap=[[zgp[:].ap[0][0], 128], [65, 4]]))

        if DEBUG:
            dg = sb.tile([128, 640], F32, tag="dbg4", name="dbg4")
            nc.vector.tensor_copy(dg[:], g_all[0][:, 0:640])
            nc.sync.dma_start(dbg["d_gall"], dg[:])
            nc.sync.dma_start(dbg["d_t16"], t16[:])
            nc.sync.dma_start(dbg["d_p16"], p16_sb[2][:])
            nc.sync.dma_start(dbg["d_Gcat"], G_cat[:])
            nc.sync.dma_start(dbg["d_xw0"], xw_sb[(2, 0)][:])
            nc.sync.dma_start(dbg["d_xw1"], xw_sb[(2, 1)][:])
        # last scheduled scale's BN stats (cols 260:325 of G_cat)
        for t in range(2):
            zg = (pgx if t == 0 else pcv).tile(
                [128, 512], F32, tag="gx" if t == 0 else "cv", name="zgL")
            nc.tensor.matmul(zg[:, 0:65], zwtf_sb[:, 128 * t:128 * t + 128],
                             G_cat[:, 260:325], start=True, stop=True)
            zzl = p2.tile([128, 65], F32, tag="zzL", name="zzL")
            nc.vector.tensor_tensor(zzl[:], zg[:, 0:65],
                                    zw65g_sb[:, 325 * t + 260:325 * t + 325],
                                    op=AluOpType.mult)
            nc.vector.tensor_reduce(arin_sb[:, 10 * t + 4:10 * t + 5],
                                    zzl[:], AXX, AluOpType.add)
            nc.vector.tensor_copy(arin_sb[:, 10 * t + 9:10 * t + 10],
                                  zg[:, 64:65])

        if DEBUG:
            nc.sync.dma_start(dbg["d_arin"], arin_sb[:])
        # ---------------- stats AllGather ----------------
        arin = dram.tile([128, 20], F32, name="arin_d")
        arout = dram.tile([128 * NCORES, 20], F32, name="arout_d")
        nc.sync.dma_start(arin[:], arin_sb[:])
        nc.gpsimd.collective_compute(
            "AllGather", AluOpType.bypass,
            replica_groups=[list(range(NCORES))],
            ins=[arin.opt()], outs=[arout.opt()])
        gath = sb.tile([128, 160], F32, tag="gath", name="gath")
        src = bass.AP(tensor=arout[:].tensor, offset=arout[:].offset,
                      ap=[[20, 128], [2560, 8], [1, 20]])
        nc.sync.dma_start(gath[:], src)
        stats = sb.tile([128, 20], F32, tag="stats", name="stats")
        nc.vector.tensor_reduce(
            stats[:],
            bass.AP(tensor=gath[:].tensor, offset=gath[:].offset,
                    ap=[[gath[:].ap[0][0], 128], [1, 20], [20, 8]]),
            AXX, AluOpType.add)

        if DEBUG:
            nc.sync.dma_start(dbg["d_stats"], stats[:])
        # ---------------- BN coefficients (SCHED order) ----------------
        a16 = sb.tile([128, 10], F16, tag="a16", name="a16")
        bacc_t = [sb.tile([128, 1], F32, tag=f"bacc{t}", name=f"bacc{t}")
                  for t in range(2)]
        for t in range(2):
            s2v = stats[:, 10 * t:10 * t + 5]
            s1v = stats[:, 10 * t + 5:10 * t + 10]
            mean = p2.tile([128, 5], F32, tag="mean", name="mean")
            nc.vector.tensor_scalar_mul(mean[:], s1v, 1.0 / NPIX)
            m2 = p2.tile([128, 5], F32, tag="m2", name="m2")
            nc.vector.tensor_tensor(m2[:], mean[:], mean[:], op=AluOpType.mult)
            var = p2.tile([128, 5], F32, tag="var", name="var")
            nc.vector.scalar_tensor_tensor(var[:], s2v, 1.0 / NPIX, m2[:],
                                           op0=AluOpType.mult,
                                           op1=AluOpType.subtract)
            sq = p2.tile([128, 5], F32, tag="sq", name="sq")
            nc.scalar.activation(sq[:], var[:], AFT.Sqrt,
                                 bias=gmp_sb[:, 12:13])
            rinv = p2.tile([128, 5], F32, tag="rinv", name="rinv")
            nc.vector.reciprocal_approx_fast(rinv[:], sq[:])
            af = p2.tile([128, 5], F32, tag="af", name="af")
            nc.vector.tensor_tensor(af[:], rinv[:], gmp_sb[:, 5 * t:5 * t + 5],
                                    op=AluOpType.mult)
            nc.vector.tensor_copy(a16[:, 5 * t:5 * t + 5], af[:])
            tmb = p2.tile([128, 5], F32, tag="tmb", name="tmb")
            nc.vector.tensor_tensor(tmb[:], af[:], mean[:], op=AluOpType.mult)
            tmbr = p2.tile([128, 1], F32, tag="tmbr", name="tmbr")
            nc.vector.tensor_reduce(tmbr[:], tmb[:], AXX, AluOpType.add)
            nc.vector.tensor_tensor(bacc_t[t][:], gmp_sb[:, 10 + t:11 + t],
                                    tmbr[:], op=AluOpType.subtract)
        # a5cat rows via PE transposes: one accumulation group per bank
        # (start zeroes the bank; disjoint-column matmuls add into zeros),
        # then 3 wide copies instead of 10 narrow ones
        a5cat = sb.tile([1, 1280], F16, tag="a5cat", name="a5cat")
        banks = [(pgx, "gx", 0, 4), (pcv, "cv", 4, 8), (psc, "sc", 8, 10)]
        for pool, tag, i0, i1 in banks:
            atp = pool.tile([128, 512], F32, tag=tag, name="tp")
            for ii in range(i0, i1):
                si, t = ii // 2, ii % 2
                nc.tensor.matmul(
                    atp[0:1, 128 * (ii - i0):128 * (ii - i0) + 128],
                    a16[:, 5 * t + si:5 * t + si + 1],
                    i128_sb[:], start=(ii == i0), stop=(ii == i1 - 1))
            nc.vector.tensor_copy(a5cat[0:1, 128 * i0:128 * i1],
                                  atp[0:1, 0:128 * (i1 - i0)])
        W_sb = []
        for si in range(5):
            abp = (pcv if si % 2 else pgx).tile(
                [128, 512], F32, tag="cv" if si % 2 else "gx", name="ab")
            nc.tensor.matmul(abp[:, 0:256], ones128[:],
                             a5cat[0:1, 256 * si:256 * si + 256],
                             start=True, stop=True)
            W = sb.tile([128, 256], F16, tag=f"W{si}", name=f"W{si}")
            nc.vector.tensor_tensor(W[:], zwt16_sb[:], abp[:, 0:256],
                                    op=AluOpType.mult)
            W_sb.append(W)

        if DEBUG:
            nc.sync.dma_start(dbg["d_a16"], a16[:])
            nc.sync.dma_start(dbg["d_W0"], W_sb[0][:])
        # ---------------- final matmul + store ----------------
        for t in range(2):
            out_sb = sb.tile([128, QL], F32, tag=f"osb{t}", name=f"osb{t}")
            for h in range(2):
                for par in range(2):
                    nb = 5 if par == 0 else 4
                    fp = psc.tile([128, 576], F32, tag="sc", name="sc")
                    for si in range(5):
                        nc.tensor.matmul(
                            fp[:, 0:64 * nb],
                            W_sb[si][64 * par:64 * par + 64,
                                     128 * t:128 * t + 128],
                            xw_sb[(SCHED[si], h)][64 * par:64 * par + 64,
                                                  0:64 * nb],
                            start=(si == 0), stop=(si == 4))
                    dst = bass.AP(
                        tensor=out_sb[:].tensor,
                        offset=out_sb[:].offset + 64 * (9 * h + par),
                        ap=[[out_sb[:].ap[0][0], 128], [128, nb], [1, 64]])
                    nc.vector.tensor_scalar_add(
                        dst,
                        fp[:].rearrange("p (b c) -> p b c", c=64)[:, 0:nb, :],
                        bacc_t[t][:])
                nc.sync.dma_start(
                    out_d[:, QL * t + 576 * h:QL * t + 576 * h + 576],
                    out_sb[:, 576 * h:576 * h + 576])

    nc.compile()
    return nc


def kernel(**inputs):
    f32, f16 = np.float32, np.float16
    persp = np.asarray(inputs['perspective'], dtype=f32)
    t_w = np.asarray(inputs['t_w'], dtype=f32)
    z_w = np.asarray(inputs['z_w'], dtype=f32)
    if 'nc' not in _CACHED:
        _CACHED['nc'] = build()
    nc = _CACHED['nc']
    KT = [max(1, c // 128) for c in CR]

    # local query order: col L = 64*j + q  ->  global n = 36*q + 18*h + j
    Lq = np.arange(QL)
    qv, jv = Lq % 64, Lq // 64
    twt16 = np.zeros((128, 128), f16)
    twt = np.ascontiguousarray(t_w.T)
    twt16[:, 0:64] = twt[0:128].astype(f16)
    twt16[:, 64:128] = twt[128:256].astype(f16)
    zwt = np.ascontiguousarray(z_w.T)
    zw65 = np.zeros((128, 650), f32)
    for t in range(2):
        for si in range(5):
            zw65[:, 325 * t + 65 * si:325 * t + 65 * si + 64] = \
                z_w[128 * t:128 * t + 128, :]
    gmp = np.zeros((128, 13), f32)
    for t in range(2):
        for si in range(5):
            gmp[:, 5 * t + si] = np.asarray(
                inputs[f'bn{SCHED[si]}_g'], f32)[128 * t:128 * t + 128]
        gmp[:, 10 + t] = sum(np.asarray(inputs[f'bn{s}_b'], f32)
                             for s in range(5))[128 * t:128 * t + 128]
    gmp[:, 12] = EPS
    i128 = np.eye(128, dtype=f16)
    nkt = sum(KT)
    pwt = np.zeros((128, 64 * nkt), f16)
    gwt = np.zeros((128, 64 * nkt), f16)
    koff = 0
    for s in range(5):
        pw = np.asarray(inputs[f'p{s}_w'], f32).T
        gw = np.asarray(inputs[f'g{s}_w'], f32).T
        for kk in range(KT[s]):
            r0, r1 = 128 * kk, min(128 * kk + 128, CR[s])
            pwt[0:r1 - r0, koff:koff + 64] = pw[r0:r1].astype(f16)
            gwt[0:r1 - r0, koff:koff + 64] = gw[r0:r1].astype(f16)
            koff += 64

    in_maps = []
    for i in range(4):
        for h in range(2):
            nglob = 36 * qv + 18 * h + jv
            pi = persp[i].reshape(CP, 2304)[:, nglob]
            p16 = np.zeros((128, 2 * QL), f16)
            p16[:, 0:QL] = pi[0:128].astype(f16)
            p16[:, QL:] = pi[128:256].astype(f16)
            m = {"persp": p16, "twt": twt16,
                 "zwt16": np.concatenate([zwt, zwt], axis=0).astype(f16),
                 "zwtf": zwt, "zw65": zw65, "gmp": gmp, "i128": i128,
                 "pwt": pwt, "gwt": gwt}
            for s in range(5):
                rs = np.asarray(inputs[f'response{s}'], f32)[i].reshape(CR[s], MS[s])
                rt = np.zeros((min(CR[s], 128), KT[s] * MS[s]), f16)
                for kk in range(KT[s]):
                    r0, r1 = 128 * kk, min(128 * kk + 128, CR[s])
                    rt[0:r1 - r0, MS[s] * kk:MS[s] * kk + MS[s]] = \
                        rs[r0:r1].astype(f16)
                m[f"resp{s}"] = rt
            in_maps.append(m)
    res = bass_utils.run_bass_kernel_spmd(nc, in_maps,
                                          core_ids=list(range(NCORES)))
    _CACHED['res'] = res
    out = np.zeros((4, CP, 2304), np.float32)
    for i in range(4):
        for h in range(2):
            o = res.results[i * 2 + h]["out"]
            full = np.concatenate([o[:, 0:QL], o[:, QL:]], axis=0)
            out[i][:, QL * h:QL * h + QL] = full
    return out.reshape(4, CP, 48, 48)


if __name__ == "__main__":
    from concourse.timeline_sim import TimelineSim
    nc = build()
    tl = TimelineSim(nc, trace=False)
    print(f"TimelineSim: {tl.simulate():.0f} ns")


# revision 34
# speedup vs baseline: 1.0180x; 1.0074x over previous
"""Trainium2 Bass kernel for nn_CNL_5 (5-scale context non-local block).

Sharding: 8 cores = 4 samples x 2 query-subsets. Local query column order is
L = 64*j + q  (q = z-conv input channel = n//36-block, j = n%18), chosen so the
xbar DMA transpose (out[p,b,c] = in[c,128b+p]) directly yields the z-conv
operand x[q, pixel] with j-parity split across partition halves. outT is
padded to 640-col halves so each query-half transposes independently.

BN batch stats travel as per-channel (s1, s2) quadratic forms [128,20] through
one small AllGather; heavy math is fp16/bf16 on the PE at 1 cyc/row.
"""
import numpy as np
import ml_dtypes
from contextlib import ExitStack

import concourse.bass as bass
import concourse.bacc as bacc
import concourse.tile as tile
from concourse import mybir
from concourse import bass_utils
from concourse.alu_op_type import AluOpType

F32 = mybir.dt.float32
F16 = mybir.dt.float16
BF16 = mybir.dt.bfloat16
AFT = mybir.ActivationFunctionType
AXX = mybir.AxisListType.X

NCORES = 8
CP = 256
QL = 1152
CR = [64, 256, 512, 1024, 2048]
MS = [2304, 2304, 576, 144, 36]
CSH = [0.0, 10.0, 15.0, 25.0, 40.0]
EPS = 1e-5
NPIX = 4 * 2304.0
SCHED = [4, 2, 3, 1, 0]
DEBUG = False

_CACHED = {}


def mtiles(M):
    out, off = [], 0
    while off < M:
        w = min(128, M - off)
        out.append((off, w))
        off += w
    return out


def chunks512(N):
    out, off = [], 0
    while off < N:
        w = min(512, N - off)
        out.append((off, w))
        off += w
    return out


def build():
    nc = bacc.Bacc("TRN2", target_bir_lowering=False, debug=False,
                   num_devices=NCORES)
    KT = [len(mtiles(c)) for c in CR]
    persp_d = nc.dram_tensor("persp", [128, 2 * QL], F16, kind="ExternalInput").ap()
    twt_d = nc.dram_tensor("twt", [128, 128], F16, kind="ExternalInput").ap()
    resp_d = [nc.dram_tensor(f"resp{s}", [min(CR[s], 128), KT[s] * MS[s]], F16,
                             kind="ExternalInput").ap() for s in range(5)]
    pwt_d = nc.dram_tensor("pwt", [128, 64 * sum(KT)], F16, kind="ExternalInput").ap()
    gwt_d = nc.dram_tensor("gwt", [128, 64 * sum(KT)], F16, kind="ExternalInput").ap()
    zwt16_d = nc.dram_tensor("zwt16", [128, CP], F16, kind="ExternalInput").ap()
    zwtf_d = nc.dram_tensor("zwtf", [64, CP], F32, kind="ExternalInput").ap()
    zw65_d = nc.dram_tensor("zw65", [128, 650], F32, kind="ExternalInput").ap()
    gmp_d = nc.dram_tensor("gmp", [128, 13], F32, kind="ExternalInput").ap()
    i128_d = nc.dram_tensor("i128", [128, 128], F16, kind="ExternalInput").ap()
    out_d = nc.dram_tensor("out", [128, 2 * QL], F32, kind="ExternalOutput").ap()
    dbg = {}
    if DEBUG:
        for nm, shp, dt in [("d_t16", [64, QL], F16), ("d_p16", [64, MS[2]], F16),
                            ("d_outT", [64, 1280], F16), ("d_xw0", [128, 320], F16),
                            ("d_xw1", [128, 320], F16), ("d_Gcat", [64, 325], F32),
                            ("d_arin", [128, 20], F16), ("d_stats", [128, 20], F32),
                            ("d_a16", [128, 10], F16), ("d_W0", [128, 256], F16),
                            ("d_opn", [64, 576], F32), ("d_opd", [64, 576], F32),
                            ("d_rc", [64, 576], F32), ("d_et", [128, 576], F16),
                            ("d_gall", [128, 640], F32)]:
            dbg[nm] = nc.dram_tensor(nm, shp, dt, kind="ExternalOutput").ap()

    with tile.TileContext(nc) as tc, ExitStack() as ctx:
        sb = ctx.enter_context(tc.tile_pool(name="sb", bufs=1))
        p2 = ctx.enter_context(tc.tile_pool(name="p2", bufs=2))
        et3 = ctx.enter_context(tc.tile_pool(name="et3", bufs=3))
        p16p = ctx.enter_context(tc.tile_pool(name="p16p", bufs=3))
        dram = ctx.enter_context(tc.tile_pool(name="dram", bufs=1, space="DRAM"))
        psc = ctx.enter_context(tc.tile_pool(name="psc", bufs=2, space="PSUM"))
        pop = ctx.enter_context(tc.tile_pool(name="pop", bufs=1, space="PSUM"))
        pcv = ctx.enter_context(tc.tile_pool(name="pcv", bufs=1, space="PSUM"))
        pgx = ctx.enter_context(tc.tile_pool(name="pgx", bufs=1, space="PSUM"))

        # ---------------- static loads ----------------
        twt_sb = sb.tile([128, 128], F16, tag="twt", name="twt")
        nc.sync.dma_start(twt_sb[:], twt_d)
        persp_sb = sb.tile([128, 2 * QL], F16, tag="persp", name="persp")
        nc.sync.dma_start(persp_sb[:], persp_d)
        pwt_sb = sb.tile([128, 64 * sum(KT)], F16, tag="pwt", name="pwt")
        nc.sync.dma_start(pwt_sb[:], pwt_d)
        resp_sb = [None] * 5
        s0_ = SCHED[0]
        r = sb.tile([min(CR[s0_], 128), KT[s0_] * MS[s0_]], F16,
                    tag=f"resp{s0_}", name=f"resp{s0_}")
        nc.sync.dma_start(r[:], resp_d[s0_])
        resp_sb[s0_] = r
        gwt_sb = sb.tile([128, 64 * sum(KT)], F16, tag="gwt", name="gwt")
        nc.sync.dma_start(gwt_sb[:], gwt_d)
        for s in SCHED[1:]:
            r = sb.tile([min(CR[s], 128), KT[s] * MS[s]], F16,
                        tag=f"resp{s}", name=f"resp{s}")
            nc.sync.dma_start(r[:], resp_d[s])
            resp_sb[s] = r
        KOFF = [64 * sum(KT[:s]) for s in range(5)]
        zwt16_sb = sb.tile([128, CP], F16, tag="zwt16", name="zwt16")
        nc.sync.dma_start(zwt16_sb[:], zwt16_d)
        zwtf_sb = sb.tile([64, CP], F32, tag="zwtf", name="zwtf")
        nc.sync.dma_start(zwtf_sb[:], zwtf_d)
        zw65g_sb = sb.tile([128, 650], F32, tag="zw65", name="zw65")
        nc.sync.dma_start(zw65g_sb[:], zw65_d)
        gmp_sb = sb.tile([128, 13], F32, tag="gmp", name="gmp")
        nc.sync.dma_start(gmp_sb[:], gmp_d)
        i128_sb = sb.tile([128, 128], F16, tag="i128", name="i128")
        nc.sync.dma_start(i128_sb[:], i128_d)
        bias_sb = []
        for s in range(5):
            bt = sb.tile([128, 1], F32, tag=f"bias{s}", name=f"bias{s}")
            nc.vector.memset(bt[:], -CSH[s])
            bias_sb.append(bt)
        ones128 = sb.tile([1, 128], F16, tag="ones128", name="ones128")
        nc.vector.memset(ones128[:], 1.0)
        onesc = sb.tile([64, 1], F16, tag="onesc", name="onesc")
        nc.vector.memset(onesc[:], 1.0)
        g_all = [sb.tile([128, 128 * 18], BF16, tag=f"gall{i}", name=f"gall{i}")
                 for i in range(3)]
        for i in range(3):
            nc.gpsimd.memset(
                g_all[i][:].rearrange("p (k c) -> p k c", c=128)[:, :, 64:128], 1.0)
        # outT ring: pre-zero the 64-col pads of both ring slots
        oT = [p2.tile([64, 1280], F16, tag="outT", name=f"outTz{i}")
              for i in range(2)]
        for i in range(2):
            for h in range(2):
                nc.gpsimd.memset(oT[i][:, 640 * h + 576:640 * h + 640], 0.0)

        # ---------------- t conv: t16 [64, QL] ----------------
        t16 = sb.tile([64, QL], F16, tag="t16", name="t16")
        for off, w in chunks512(QL):
            tp = pgx.tile([128, 512], F32, tag="gx", name="gx")
            for kk in range(2):
                nc.tensor.matmul(tp[0:64, 0:w], twt_sb[:, 64 * kk:64 * kk + 64],
                                 persp_sb[:, QL * kk + off:QL * kk + off + w],
                                 start=(kk == 0), stop=(kk == 1))
            nc.vector.tensor_copy(t16[:, off:off + w], tp[0:64, 0:w])

        # ---------------- per-scale p/g convs (as deferrable units) --------
        p16_sb, xw_sb = {}, {}

        def conv_units(s, evac_eng):
            nct = mtiles(CR[s])
            p16 = p16p.tile([64, MS[s]], F16, tag="p16", name=f"p16_{s}")
            p16_sb[s] = p16
            units = []

            def p_unit(off, w):
                def emit():
                    pp = pcv.tile([128, 512], F32, tag="cv", name="cv")
                    for kk, (co, cw) in enumerate(nct):
                        nc.tensor.matmul(
                            pp[0:64, 0:w],
                            pwt_sb[0:cw, KOFF[s] + 64 * kk:KOFF[s] + 64 * kk + 64],
                            resp_sb[s][0:cw, MS[s] * kk + off:MS[s] * kk + off + w],
                            start=(kk == 0), stop=(kk == len(nct) - 1))
                    evac_eng.tensor_copy(p16[:, off:off + w], pp[0:64, 0:w])
                return emit

            def g_unit(b0, batch):
                def emit():
                    ga = g_all[SCHED.index(s) % 3]
                    gp = pcv.tile([128, 512], F32, tag="cv", name="cv")
                    for k, (moff, mw) in enumerate(batch):
                        for kk, (co, cw) in enumerate(nct):
                            nc.tensor.matmul(
                                gp[0:mw, 64 * k:64 * k + 64],
                                resp_sb[s][0:cw, MS[s] * kk + moff:MS[s] * kk + moff + mw],
                                gwt_sb[0:cw, KOFF[s] + 64 * kk:KOFF[s] + 64 * kk + 64],
                                start=(kk == 0), stop=(kk == len(nct) - 1))
                    dst = ga[:].rearrange("p (k c) -> p k c", c=128)[
                        :, b0:b0 + len(batch), 0:64]
                    src = gp[:].rearrange("p (k c) -> p k c", c=64)[
                        :, 0:len(batch), :]
                    evac_eng.tensor_copy(dst, src)
                return emit

            for off, w in chunks512(MS[s]):
                units.append((s, p_unit(off, w)))
            mts = mtiles(MS[s])
            for b0 in range(0, len(mts), 8):
                units.append((s, g_unit(b0, mts[b0:b0 + 8])))
            return units

        for _, u in conv_units(SCHED[0], nc.vector):
            u()
        pending = conv_units(SCHED[1], nc.vector)

        # ---------------- attention per scale ----------------
        arin_sb = sb.tile([128, 20], F16, tag="arin", name="arin")
        G_cat = sb.tile([64, 325], F32, tag="G_cat", name="G_cat")
        for si, s in enumerate(SCHED):
            mts = mtiles(MS[s])
            ga = g_all[si % 3]
            while pending and pending[0][0] == s:
                pending.pop(0)[1]()
            if si + 2 < 5:
                pending += conv_units(SCHED[si + 2], nc.vector)
            outT = p2.tile([64, 1280], F16, tag="outT", name=f"outT{s}")
            gm = pgx.tile([128, 512], F32, tag="gx", name="gx")
            for h in range(2):
                op = pop.tile([128, 576], F32, tag="op", name="op")
                for k, (moff, mw) in enumerate(mts):
                    sc = psc.tile([128, 576], F32, tag="sc", name="sc")
                    for co, cw in ((0, 512), (512, 64)):
                        nc.tensor.matmul(
                            sc[0:mw, co:co + cw],
                            p16_sb[s][:, moff:moff + mw],
                            t16[:, 576 * h + co:576 * h + co + cw],
                            start=True, stop=True)
                    et = et3.tile([128, 576], BF16, tag="et", name="et")
                    nc.scalar.activation(et[0:mw, :], sc[0:mw, :], AFT.Exp,
                                         bias=bias_sb[s][0:mw, :])
                    if DEBUG and si == 0 and h == 0 and k == 0:
                        de = sb.tile([128, 576], F16, tag="dbg3", name="dbg3")
                        nc.vector.tensor_copy(de[:], et[:])
                        nc.sync.dma_start(dbg["d_et"], de[:])
                    for co, cw in ((0, 512), (512, 64)):
                        nc.tensor.matmul(
                            op[:, co:co + cw],
                            ga[0:mw, 128 * k:128 * k + 128],
                            et[0:mw, co:co + cw],
                            start=(k == 0), stop=(k == len(mts) - 1))
                    if pending and (k % 2 == 1 or len(mts) < 10):
                        pending.pop(0)[1]()
                rc = p2.tile([64, 576], F32, tag="rc", name="rc")
                if DEBUG and si == 0 and h == 1:
                    dn = sb.tile([64, 576], F32, tag="dbg1", name="dbg1")
                    nc.vector.tensor_copy(dn[:], op[0:64, :])
                    nc.sync.dma_start(dbg["d_opn"], dn[:])
                    dd = sb.tile([64, 576], F32, tag="dbg2", name="dbg2")
                    nc.vector.tensor_copy(dd[:], op[64:128, :])
                    nc.sync.dma_start(dbg["d_opd"], dd[:])
                nc.vector.reciprocal(rc[:], op[64:128, :])
                if DEBUG and si == 0 and h == 1:
                    nc.sync.dma_start(dbg["d_rc"], rc[:])
                if len(mts) < 10:
                    # small scales: Act idles at the boundary — copy the
                    # numerator out on Act in parallel with the reciprocal so
                    # the op PSUM ring frees ~2x faster for the next half
                    num = p2.tile([64, 576], F32, tag="num", name="num")
                    nc.scalar.activation(num[:], op[0:64, :], AFT.Copy)
                    nc.vector.tensor_tensor(outT[:, 640 * h:640 * h + 576],
                                            num[:], rc[:], op=AluOpType.mult)
                else:
                    nc.vector.tensor_tensor(outT[:, 640 * h:640 * h + 576],
                                            op[0:64, :], rc[:],
                                            op=AluOpType.mult)
                # per-half xbar transpose + svec + gram
                xw = sb.tile([128, 5 * 64], F16, tag=f"xw{s}_{h}",
                             name=f"xw{s}_{h}")
                xw_sb[(s, h)] = xw
                nc.sync.dma_start_transpose(
                    xw[:].rearrange("p (b c) -> p b c", c=64),
                    outT[:, 640 * h:640 * h + 640])
                for j in range(9):
                    nc.tensor.matmul(
                        gm[0:64, 0:64],
                        outT[:, 640 * h + 64 * j:640 * h + 64 * j + 64],
                        outT[:, 640 * h + 64 * j:640 * h + 64 * j + 64],
                        start=(h == 0 and j == 0), stop=(h == 1 and j == 8))
            if DEBUG and si == 0:
                nc.sync.dma_start(dbg["d_outT"], outT[:])
            # order in one bank: gram -> G-copy -> svec-sum -> svrow-copy ->
            # transpose -> G-col copy (chained via overlapping regions, since
            # a matmul 'start' resets the whole bank)
            nc.vector.tensor_copy(G_cat[:, 65 * si:65 * si + 64],
                                  gm[0:64, 0:64])
            for h in range(2):
                for j in range(9):
                    nc.tensor.matmul(
                        gm[0:1, 0:64], onesc[:],
                        outT[:, 640 * h + 64 * j:640 * h + 64 * j + 64],
                        start=(h == 0 and j == 0), stop=(h == 1 and j == 8))
            svrow = p2.tile([1, 64], F16, tag="svrow", name="svrow")
            nc.vector.tensor_copy(svrow[:], gm[0:1, 0:64])
            nc.tensor.matmul(gm[0:64, 0:1], svrow[:], onesc[0:1, 0:1],
                             start=True, stop=True)
            nc.vector.tensor_copy(G_cat[:, 65 * si + 64:65 * si + 65],
                                  gm[0:64, 0:1])
            if si == 3:
                # BN partials for first 4 scheduled scales (off critical path)
                for t in range(2):
                    zgp = (pgx if t == 0 else pcv).tile(
                        [128, 512], F32, tag="gx" if t == 0 else "cv",
                        name="zgP")
                    nc.tensor.matmul(zgp[:, 0:260],
                                     zwtf_sb[:, 128 * t:128 * t + 128],
                                     G_cat[:, 0:260], start=True, stop=True)
                    zzp = p2.tile([128, 260], F32, tag="zz", name="zz")
                    nc.vector.tensor_tensor(
                        zzp[:], zgp[:, 0:260],
                        zw65g_sb[:, 325 * t:325 * t + 260],
                        op=AluOpType.mult)
                    s2p = p2.tile([128, 4], F32, tag="s2p", name="s2p")
                    nc.vector.tensor_reduce(
                        s2p[:],
                        zzp[:].rearrange("p (s c) -> p s c", c=65), AXX,
                        AluOpType.add)
                    nc.vector.tensor_copy(arin_sb[:, 10 * t:10 * t + 4],
                                          s2p[:])
                    nc.vector.tensor_copy(
                        arin_sb[:, 10 * t + 5:10 * t + 9],
                        bass.AP(tensor=zgp[:].tensor,
                                offset=zgp[:].offset + 64,
                                ap=[[zgp[:].ap[0][0], 128], [65, 4]]))

        if DEBUG:
            dg = sb.tile([128, 640], F32, tag="dbg4", name="dbg4")
            nc.vector.tensor_copy(dg[:], g_all[0][:, 0:640])
            nc.sync.dma_start(dbg["d_gall"], dg[:])
            nc.sync.dma_start(dbg["d_t16"], t16[:])
            nc.sync.dma_start(dbg["d_p16"], p16_sb[2][:])
            nc.sync.dma_start(dbg["d_Gcat"], G_cat[:])
            nc.sync.dma_start(dbg["d_xw0"], xw_sb[(2, 0)][:])
            nc.sync.dma_start(dbg["d_xw1"], xw_sb[(2, 1)][:])
        # last scheduled scale's BN stats (cols 260:325 of G_cat)
        for t in range(2):
            zg = (pgx if t == 0 else pcv).tile(
                [128, 512], F32, tag="gx" if t == 0 else "cv", name="zgL")
            nc.tensor.matmul(zg[:, 0:65], zwtf_sb[:, 128 * t:128 * t + 128],
                             G_cat[:, 260:325], start=True, stop=True)
            zzl = p2.tile([128, 65], F32, tag="zzL", name="zzL")
            nc.vector.tensor_tensor(zzl[:], zg[:, 0:65],
                                    zw65g_sb[:, 325 * t + 260:325 * t + 325],
                                    op=AluOpType.mult)
            s2l = p2.tile([128, 1], F32, tag="s2l", name="s2l")
            nc.vector.tensor_reduce(s2l[:], zzl[:], AXX, AluOpType.add)
            nc.vector.tensor_copy(arin_sb[:, 10 * t + 4:10 * t + 5], s2l[:])
            nc.vector.tensor_copy(arin_sb[:, 10 * t + 9:10 * t + 10],
                                  zg[:, 64:65])

        if DEBUG:
            nc.sync.dma_start(dbg["d_arin"], arin_sb[:])
        # ---------------- stats AllGather ----------------
        arin = dram.tile([128, 20], F16, name="arin_d")
        arout = dram.tile([128 * NCORES, 20], F16, name="arout_d")
        nc.sync.dma_start(arin[:], arin_sb[:])
        nc.gpsimd.collective_compute(
            "AllGather", AluOpType.bypass,
            replica_groups=[list(range(NCORES))],
            ins=[arin.opt()], outs=[arout.opt()])
        gath = sb.tile([128, 160], F16, tag="gath", name="gath")
        src = bass.AP(tensor=arout[:].tensor, offset=arout[:].offset,
                      ap=[[20, 128], [2560, 8], [1, 20]])
        nc.sync.dma_start(gath[:], src)
        stats = sb.tile([128, 20], F32, tag="stats", name="stats")
        nc.vector.tensor_reduce(
            stats[:],
            bass.AP(tensor=gath[:].tensor, offset=gath[:].offset,
                    ap=[[gath[:].ap[0][0], 128], [1, 20], [20, 8]]),
            AXX, AluOpType.add)

        if DEBUG:
            nc.sync.dma_start(dbg["d_stats"], stats[:])
        # ---------------- BN coefficients (SCHED order) ----------------
        a16 = sb.tile([128, 10], F16, tag="a16", name="a16")
        bacc_t = [sb.tile([128, 1], F32, tag=f"bacc{t}", name=f"bacc{t}")
                  for t in range(2)]
        for t in range(2):
            s2v = stats[:, 10 * t:10 * t + 5]
            s1v = stats[:, 10 * t + 5:10 * t + 10]
            mean = p2.tile([128, 5], F32, tag="mean", name="mean")
            nc.vector.tensor_scalar_mul(mean[:], s1v, 1.0 / NPIX)
            m2 = p2.tile([128, 5], F32, tag="m2", name="m2")
            nc.vector.tensor_tensor(m2[:], mean[:], mean[:], op=AluOpType.mult)
            var = p2.tile([128, 5], F32, tag="var", name="var")
            nc.vector.scalar_tensor_tensor(var[:], s2v, 1.0 / NPIX, m2[:],
                                           op0=AluOpType.mult,
                                           op1=AluOpType.subtract)
            sq = p2.tile([128, 5], F32, tag="sq", name="sq")
            nc.scalar.activation(sq[:], var[:], AFT.Sqrt,
                                 bias=gmp_sb[:, 12:13])
            rinv = p2.tile([128, 5], F32, tag="rinv", name="rinv")
            nc.vector.reciprocal_approx_fast(rinv[:], sq[:])
            af = p2.tile([128, 5], F32, tag="af", name="af")
            nc.vector.tensor_tensor(af[:], rinv[:], gmp_sb[:, 5 * t:5 * t + 5],
                                    op=AluOpType.mult)
            nc.vector.tensor_copy(a16[:, 5 * t:5 * t + 5], af[:])
            tmb = p2.tile([128, 5], F32, tag="tmb", name="tmb")
            nc.vector.tensor_tensor(tmb[:], af[:], mean[:], op=AluOpType.mult)
            tmbr = p2.tile([128, 1], F32, tag="tmbr", name="tmbr")
            nc.vector.tensor_reduce(tmbr[:], tmb[:], AXX, AluOpType.add)
            nc.vector.tensor_tensor(bacc_t[t][:], gmp_sb[:, 10 + t:11 + t],
                                    tmbr[:], op=AluOpType.subtract)
        # a5cat rows via PE transposes: one accumulation group per bank
        # (start zeroes the bank; disjoint-column matmuls add into zeros),
        # then 3 wide copies instead of 10 narrow ones
        a5cat = sb.tile([1, 1280], F16, tag="a5cat", name="a5cat")
        banks = [(pgx, "gx", 0, 4), (pcv, "cv", 4, 8), (psc, "sc", 8, 10)]
        for pool, tag, i0, i1 in banks:
            atp = pool.tile([128, 512], F32, tag=tag, name="tp")
            for ii in range(i0, i1):
                si, t = ii // 2, ii % 2
                nc.tensor.matmul(
                    atp[0:1, 128 * (ii - i0):128 * (ii - i0) + 128],
                    a16[:, 5 * t + si:5 * t + si + 1],
                    i128_sb[:], start=(ii == i0), stop=(ii == i1 - 1))
            nc.vector.tensor_copy(a5cat[0:1, 128 * i0:128 * i1],
                                  atp[0:1, 0:128 * (i1 - i0)])
        W_sb = []
        for si in range(5):
            abp = (pcv if si % 2 else pgx).tile(
                [128, 512], F32, tag="cv" if si % 2 else "gx", name="ab")
            nc.tensor.matmul(abp[:, 0:256], ones128[:],
                             a5cat[0:1, 256 * si:256 * si + 256],
                             start=True, stop=True)
            W = sb.tile([128, 256], F16, tag=f"W{si}", name=f"W{si}")
            nc.vector.tensor_tensor(W[:], zwt16_sb[:], abp[:, 0:256],
                                    op=AluOpType.mult)
            W_sb.append(W)

        if DEBUG:
            nc.sync.dma_start(dbg["d_a16"], a16[:])
            nc.sync.dma_start(dbg["d_W0"], W_sb[0][:])
        # ---------------- final matmul + store ----------------
        for t in range(2):
            out_sb = sb.tile([128, QL], F32, tag=f"osb{t}", name=f"osb{t}")
            for h in range(2):
                for par in range(2):
                    nb = 5 if par == 0 else 4
                    fp = psc.tile([128, 576], F32, tag="sc", name="sc")
                    for si in range(5):
                        nc.tensor.matmul(
                            fp[:, 0:64 * nb],
                            W_sb[si][64 * par:64 * par + 64,
                                     128 * t:128 * t + 128],
                            xw_sb[(SCHED[si], h)][64 * par:64 * par + 64,
                                                  0:64 * nb],
                            start=(si == 0), stop=(si == 4))
                    dst = bass.AP(
                        tensor=out_sb[:].tensor,
                        offset=out_sb[:].offset + 64 * (9 * h + par),
                        ap=[[out_sb[:].ap[0][0], 128], [128, nb], [1, 64]])
                    nc.vector.tensor_scalar_add(
                        dst,
                        fp[:].rearrange("p (b c) -> p b c", c=64)[:, 0:nb, :],
                        bacc_t[t][:])
                nc.sync.dma_start(
                    out_d[:, QL * t + 576 * h:QL * t + 576 * h + 576],
                    out_sb[:, 576 * h:576 * h + 576])

    nc.compile()
    return nc


def kernel(**inputs):
    f32, f16 = np.float32, np.float16
    persp = np.asarray(inputs['perspective'], dtype=f32)
    t_w = np.asarray(inputs['t_w'], dtype=f32)
    z_w = np.asarray(inputs['z_w'], dtype=f32)
    if 'nc' not in _CACHED:
        _CACHED['nc'] = build()
    nc = _CACHED['nc']
    KT = [max(1, c // 128) for c in CR]

    # local query order: col L = 64*j + q  ->  global n = 36*q + 18*h + j
    Lq = np.arange(QL)
    qv, jv = Lq % 64, Lq // 64
    twt16 = np.zeros((128, 128), f16)
    twt = np.ascontiguousarray(t_w.T)
    twt16[:, 0:64] = twt[0:128].astype(f16)
    twt16[:, 64:128] = twt[128:256].astype(f16)
    zwt = np.ascontiguousarray(z_w.T)
    zw65 = np.zeros((128, 650), f32)
    for t in range(2):
        for si in range(5):
            zw65[:, 325 * t + 65 * si:325 * t + 65 * si + 64] = \
                z_w[128 * t:128 * t + 128, :]
    gmp = np.zeros((128, 13), f32)
    for t in range(2):
        for si in range(5):
            gmp[:, 5 * t + si] = np.asarray(
                inputs[f'bn{SCHED[si]}_g'], f32)[128 * t:128 * t + 128]
        gmp[:, 10 + t] = sum(np.asarray(inputs[f'bn{s}_b'], f32)
                             for s in range(5))[128 * t:128 * t + 128]
    gmp[:, 12] = EPS
    i128 = np.eye(128, dtype=f16)
    nkt = sum(KT)
    pwt = np.zeros((128, 64 * nkt), f16)
    gwt = np.zeros((128, 64 * nkt), f16)
    koff = 0
    for s in range(5):
        pw = np.asarray(inputs[f'p{s}_w'], f32).T
        gw = np.asarray(inputs[f'g{s}_w'], f32).T
        for kk in range(KT[s]):
            r0, r1 = 128 * kk, min(128 * kk + 128, CR[s])
            pwt[0:r1 - r0, koff:koff + 64] = pw[r0:r1].astype(f16)
            gwt[0:r1 - r0, koff:koff + 64] = gw[r0:r1].astype(f16)
            koff += 64

    in_maps = []
    for i in range(4):
        for h in range(2):
            nglob = 36 * qv + 18 * h + jv
            pi = persp[i].reshape(CP, 2304)[:, nglob]
            p16 = np.zeros((128, 2 * QL), f16)
            p16[:, 0:QL] = pi[0:128].astype(f16)
            p16[:, QL:] = pi[128:256].astype(f16)
            m = {"persp": p16, "twt": twt16,
                 "zwt16": np.concatenate([zwt, zwt], axis=0).astype(f16),
                 "zwtf": zwt, "zw65": zw65, "gmp": gmp, "i128": i128,
                 "pwt": pwt, "gwt": gwt}
            for s in range(5):
                rs = np.asarray(inputs[f'response{s}'], f32)[i].reshape(CR[s], MS[s])
                rt = np.zeros((min(CR[s], 128), KT[s] * MS[s]), f16)
                for kk in range(KT[s]):
                    r0, r1 = 128 * kk, min(128 * kk + 128, CR[s])
                    rt[0:r1 - r0, MS[s] * kk:MS[s] * kk + MS[s]] = \
                        rs[r0:r1].astype(f16)
                m[f"resp{s}"] = rt
            in_maps.append(m)
    res = bass_utils.run_bass_kernel_spmd(nc, in_maps,
                                          core_ids=list(range(NCORES)))
    _CACHED['res'] = res
    out = np.zeros((4, CP, 2304), np.float32)
    for i in range(4):
        for h in range(2):
            o = res.results[i * 2 + h]["out"]
            full = np.concatenate([o[:, 0:QL], o[:, QL:]], axis=0)
            out[i][:, QL * h:QL * h + QL] = full
    return out.reshape(4, CP, 48, 48)


if __name__ == "__main__":
    from concourse.timeline_sim import TimelineSim
    nc = build()
    tl = TimelineSim(nc, trace=False)
    print(f"TimelineSim: {tl.simulate():.0f} ns")


# revision 39
# speedup vs baseline: 1.0437x; 1.0252x over previous
"""Trainium2 Bass kernel for nn_CNL_5 (5-scale context non-local block).

Sharding: 8 cores = 4 samples x 2 query-subsets. Local query column order is
L = 64*j + q  (q = z-conv input channel = n//36-block, j = n%18), chosen so the
xbar DMA transpose (out[p,b,c] = in[c,128b+p]) directly yields the z-conv
operand x[q, pixel] with j-parity split across partition halves. outT is
padded to 640-col halves so each query-half transposes independently.

BN batch stats travel as per-channel (s1, s2) quadratic forms [128,20] through
one small AllGather; heavy math is fp16/bf16 on the PE at 1 cyc/row.
"""
import numpy as np
import ml_dtypes
from contextlib import ExitStack

import concourse.bass as bass
import concourse.bacc as bacc
import concourse.tile as tile
from concourse import mybir
from concourse import bass_utils
from concourse.alu_op_type import AluOpType

F32 = mybir.dt.float32
F16 = mybir.dt.float16
BF16 = mybir.dt.bfloat16
AFT = mybir.ActivationFunctionType
AXX = mybir.AxisListType.X

NCORES = 8
CP = 256
QL = 1152
CR = [64, 256, 512, 1024, 2048]
MS = [2304, 2304, 576, 144, 36]
CSH = [0.0, 10.0, 15.0, 25.0, 40.0]
EPS = 1e-5
NPIX = 4 * 2304.0
SCHED = [4, 2, 3, 1, 0]
DEBUG = False

_CACHED = {}


def mtiles(M):
    out, off = [], 0
    while off < M:
        w = min(128, M - off)
        out.append((off, w))
        off += w
    return out


def chunks512(N):
    out, off = [], 0
    while off < N:
        w = min(512, N - off)
        out.append((off, w))
        off += w
    return out


def build():
    nc = bacc.Bacc("TRN2", target_bir_lowering=False, debug=False,
                   num_devices=NCORES)
    KT = [len(mtiles(c)) for c in CR]
    persp_d = nc.dram_tensor("persp", [128, 2 * QL], F16, kind="ExternalInput").ap()
    twt_d = nc.dram_tensor("twt", [128, 128], F16, kind="ExternalInput").ap()
    resp_d = [nc.dram_tensor(f"resp{s}", [min(CR[s], 128), KT[s] * MS[s]], F16,
                             kind="ExternalInput").ap() for s in range(5)]
    pwt_d = nc.dram_tensor("pwt", [128, 64 * sum(KT)], F16, kind="ExternalInput").ap()
    gwt_d = nc.dram_tensor("gwt", [128, 64 * sum(KT)], F16, kind="ExternalInput").ap()
    zwt16_d = nc.dram_tensor("zwt16", [128, CP], F16, kind="ExternalInput").ap()
    zwtf_d = nc.dram_tensor("zwtf", [64, CP], F32, kind="ExternalInput").ap()
    zw65_d = nc.dram_tensor("zw65", [128, 650], F32, kind="ExternalInput").ap()
    gmp_d = nc.dram_tensor("gmp", [128, 13], F32, kind="ExternalInput").ap()
    i128_d = nc.dram_tensor("i128", [128, 128], F16, kind="ExternalInput").ap()
    out_d = nc.dram_tensor("out", [128, 2 * QL], F32, kind="ExternalOutput").ap()
    dbg = {}
    if DEBUG:
        for nm, shp, dt in [("d_t16", [64, QL], F16), ("d_p16", [64, MS[2]], F16),
                            ("d_outT", [64, 1280], F16), ("d_xw0", [128, 320], F16),
                            ("d_xw1", [128, 320], F16), ("d_Gcat", [64, 325], F32),
                            ("d_arin", [128, 20], F16), ("d_stats", [128, 20], F32),
                            ("d_a16", [128, 10], F16), ("d_W0", [128, 256], F16),
                            ("d_opn", [64, 576], F32), ("d_opd", [64, 576], F32),
                            ("d_rc", [64, 576], F32), ("d_et", [128, 576], F16),
                            ("d_gall", [128, 640], F32)]:
            dbg[nm] = nc.dram_tensor(nm, shp, dt, kind="ExternalOutput").ap()

    with tile.TileContext(nc) as tc, ExitStack() as ctx:
        sb = ctx.enter_context(tc.tile_pool(name="sb", bufs=1))
        p2 = ctx.enter_context(tc.tile_pool(name="p2", bufs=2))
        et3 = ctx.enter_context(tc.tile_pool(name="et3", bufs=44))
        p16p = ctx.enter_context(tc.tile_pool(name="p16p", bufs=3))
        dram = ctx.enter_context(tc.tile_pool(name="dram", bufs=1, space="DRAM"))
        psc = ctx.enter_context(tc.tile_pool(name="psc", bufs=2, space="PSUM"))
        pop = ctx.enter_context(tc.tile_pool(name="pop", bufs=1, space="PSUM"))
        pcv = ctx.enter_context(tc.tile_pool(name="pcv", bufs=1, space="PSUM"))
        pgx = ctx.enter_context(tc.tile_pool(name="pgx", bufs=1, space="PSUM"))

        # ---------------- static loads ----------------
        twt_sb = sb.tile([128, 128], F16, tag="twt", name="twt")
        nc.sync.dma_start(twt_sb[:], twt_d)
        persp_sb = sb.tile([128, 2 * QL], F16, tag="persp", name="persp")
        nc.sync.dma_start(persp_sb[:], persp_d)
        pwt_sb = sb.tile([128, 64 * sum(KT)], F16, tag="pwt", name="pwt")
        nc.sync.dma_start(pwt_sb[:], pwt_d)
        resp_sb = [None] * 5
        s0_ = SCHED[0]
        r = sb.tile([min(CR[s0_], 128), KT[s0_] * MS[s0_]], F16,
                    tag=f"resp{s0_}", name=f"resp{s0_}")
        nc.sync.dma_start(r[:], resp_d[s0_])
        resp_sb[s0_] = r
        gwt_sb = sb.tile([128, 64 * sum(KT)], F16, tag="gwt", name="gwt")
        nc.sync.dma_start(gwt_sb[:], gwt_d)
        for s in SCHED[1:]:
            r = sb.tile([min(CR[s], 128), KT[s] * MS[s]], F16,
                        tag=f"resp{s}", name=f"resp{s}")
            nc.sync.dma_start(r[:], resp_d[s])
            resp_sb[s] = r
        KOFF = [64 * sum(KT[:s]) for s in range(5)]
        zwt16_sb = sb.tile([128, CP], F16, tag="zwt16", name="zwt16")
        nc.sync.dma_start(zwt16_sb[:], zwt16_d)
        zwtf_sb = sb.tile([64, CP], F32, tag="zwtf", name="zwtf")
        nc.sync.dma_start(zwtf_sb[:], zwtf_d)
        zw65g_sb = sb.tile([128, 650], F32, tag="zw65", name="zw65")
        nc.sync.dma_start(zw65g_sb[:], zw65_d)
        gmp_sb = sb.tile([128, 13], F32, tag="gmp", name="gmp")
        nc.sync.dma_start(gmp_sb[:], gmp_d)
        i128_sb = sb.tile([128, 128], F16, tag="i128", name="i128")
        nc.sync.dma_start(i128_sb[:], i128_d)
        bias_sb = []
        for s in range(5):
            bt = sb.tile([128, 1], F32, tag=f"bias{s}", name=f"bias{s}")
            nc.vector.memset(bt[:], -CSH[s])
            bias_sb.append(bt)
        ones128 = sb.tile([1, 128], F16, tag="ones128", name="ones128")
        nc.vector.memset(ones128[:], 1.0)
        onesc = sb.tile([64, 1], F16, tag="onesc", name="onesc")
        nc.vector.memset(onesc[:], 1.0)
        g_all = [sb.tile([128, 128 * 18], BF16, tag=f"gall{i}", name=f"gall{i}")
                 for i in range(3)]
        for i in range(3):
            nc.gpsimd.memset(
                g_all[i][:].rearrange("p (k c) -> p k c", c=128)[:, :, 64:128], 1.0)
        # outT ring: pre-zero the 64-col pads of both ring slots
        oT = [p2.tile([64, 1280], F16, tag="outT", name=f"outTz{i}")
              for i in range(2)]
        for i in range(2):
            for h in range(2):
                nc.gpsimd.memset(oT[i][:, 640 * h + 576:640 * h + 640], 0.0)

        # ---------------- t conv: t16 [64, QL] ----------------
        t16 = sb.tile([64, QL], F16, tag="t16", name="t16")
        for off, w in chunks512(QL):
            tp = pgx.tile([128, 512], F32, tag="gx", name="gx")
            for kk in range(2):
                nc.tensor.matmul(tp[0:64, 0:w], twt_sb[:, 64 * kk:64 * kk + 64],
                                 persp_sb[:, QL * kk + off:QL * kk + off + w],
                                 start=(kk == 0), stop=(kk == 1))
            nc.vector.tensor_copy(t16[:, off:off + w], tp[0:64, 0:w])

        # ---------------- per-scale p/g convs (as deferrable units) --------
        p16_sb, xw_sb = {}, {}

        def conv_units(s, evac_eng):
            nct = mtiles(CR[s])
            p16 = p16p.tile([64, MS[s]], F16, tag="p16", name=f"p16_{s}")
            p16_sb[s] = p16
            units = []

            def p_unit(off, w):
                def emit():
                    pp = pcv.tile([128, 512], F32, tag="cv", name="cv")
                    for kk, (co, cw) in enumerate(nct):
                        nc.tensor.matmul(
                            pp[0:64, 0:w],
                            pwt_sb[0:cw, KOFF[s] + 64 * kk:KOFF[s] + 64 * kk + 64],
                            resp_sb[s][0:cw, MS[s] * kk + off:MS[s] * kk + off + w],
                            start=(kk == 0), stop=(kk == len(nct) - 1))
                    evac_eng.tensor_copy(p16[:, off:off + w], pp[0:64, 0:w])
                return emit

            def g_unit(b0, batch):
                def emit():
                    ga = g_all[SCHED.index(s) % 3]
                    gp = pcv.tile([128, 512], F32, tag="cv", name="cv")
                    for k, (moff, mw) in enumerate(batch):
                        for kk, (co, cw) in enumerate(nct):
                            nc.tensor.matmul(
                                gp[0:mw, 64 * k:64 * k + 64],
                                resp_sb[s][0:cw, MS[s] * kk + moff:MS[s] * kk + moff + mw],
                                gwt_sb[0:cw, KOFF[s] + 64 * kk:KOFF[s] + 64 * kk + 64],
                                start=(kk == 0), stop=(kk == len(nct) - 1))
                    dst = ga[:].rearrange("p (k c) -> p k c", c=128)[
                        :, b0:b0 + len(batch), 0:64]
                    src = gp[:].rearrange("p (k c) -> p k c", c=64)[
                        :, 0:len(batch), :]
                    evac_eng.tensor_copy(dst, src)
                return emit

            for off, w in chunks512(MS[s]):
                units.append((s, p_unit(off, w)))
            mts = mtiles(MS[s])
            for b0 in range(0, len(mts), 8):
                units.append((s, g_unit(b0, mts[b0:b0 + 8])))
            return units

        for _, u in conv_units(SCHED[0], nc.vector):
            u()
        pending = conv_units(SCHED[1], nc.vector)

        # ---------------- attention per scale ----------------
        arin_sb = sb.tile([128, 20], F16, tag="arin", name="arin")
        G_cat = sb.tile([64, 325], F32, tag="G_cat", name="G_cat")
        for si, s in enumerate(SCHED):
            mts = mtiles(MS[s])
            ga = g_all[si % 3]
            while pending and pending[0][0] == s:
                pending.pop(0)[1]()
            if si + 2 < 5:
                pending += conv_units(SCHED[si + 2], nc.vector)
            outT = p2.tile([64, 1280], F16, tag="outT", name=f"outT{s}")
            gm = pgx.tile([128, 512], F32, tag="gx", name="gx")
            for h in range(2):
                op = pop.tile([128, 576], F32, tag="op", name="op")
                for k, (moff, mw) in enumerate(mts):
                    sc = psc.tile([128, 576], F32, tag="sc", name="sc")
                    for co, cw in ((0, 512), (512, 64)):
                        nc.tensor.matmul(
                            sc[0:mw, co:co + cw],
                            p16_sb[s][:, moff:moff + mw],
                            t16[:, 576 * h + co:576 * h + co + cw],
                            start=True, stop=True)
                    et = et3.tile([128, 576], BF16, tag="et", name="et")
                    nc.scalar.activation(et[0:mw, :], sc[0:mw, :], AFT.Exp,
                                         bias=bias_sb[s][0:mw, :])
                    if DEBUG and si == 0 and h == 0 and k == 0:
                        de = sb.tile([128, 576], F16, tag="dbg3", name="dbg3")
                        nc.vector.tensor_copy(de[:], et[:])
                        nc.sync.dma_start(dbg["d_et"], de[:])
                    for co, cw in ((0, 512), (512, 64)):
                        nc.tensor.matmul(
                            op[:, co:co + cw],
                            ga[0:mw, 128 * k:128 * k + 128],
                            et[0:mw, co:co + cw],
                            start=(k == 0), stop=(k == len(mts) - 1))
                    if pending and (k % 2 == 1 or len(mts) < 10):
                        pending.pop(0)[1]()
                rc = p2.tile([64, 576], F32, tag="rc", name="rc")
                if DEBUG and si == 0 and h == 1:
                    dn = sb.tile([64, 576], F32, tag="dbg1", name="dbg1")
                    nc.vector.tensor_copy(dn[:], op[0:64, :])
                    nc.sync.dma_start(dbg["d_opn"], dn[:])
                    dd = sb.tile([64, 576], F32, tag="dbg2", name="dbg2")
                    nc.vector.tensor_copy(dd[:], op[64:128, :])
                    nc.sync.dma_start(dbg["d_opd"], dd[:])
                nc.vector.reciprocal(rc[:], op[64:128, :])
                if DEBUG and si == 0 and h == 1:
                    nc.sync.dma_start(dbg["d_rc"], rc[:])
                if len(mts) < 10:
                    # small scales: Act idles at the boundary — copy the
                    # numerator out on Act in parallel with the reciprocal so
                    # the op PSUM ring frees ~2x faster for the next half
                    num = p2.tile([64, 576], F32, tag="num", name="num")
                    nc.scalar.activation(num[:], op[0:64, :], AFT.Copy)
                    nc.vector.tensor_tensor(outT[:, 640 * h:640 * h + 576],
                                            num[:], rc[:], op=AluOpType.mult)
                else:
                    nc.vector.tensor_tensor(outT[:, 640 * h:640 * h + 576],
                                            op[0:64, :], rc[:],
                                            op=AluOpType.mult)
                # per-half xbar transpose + svec + gram
                xw = sb.tile([128, 5 * 64], F16, tag=f"xw{s}_{h}",
                             name=f"xw{s}_{h}")
                xw_sb[(s, h)] = xw
                nc.sync.dma_start_transpose(
                    xw[:].rearrange("p (b c) -> p b c", c=64),
                    outT[:, 640 * h:640 * h + 640])
                for j in range(9):
                    nc.tensor.matmul(
                        gm[0:64, 0:64],
                        outT[:, 640 * h + 64 * j:640 * h + 64 * j + 64],
                        outT[:, 640 * h + 64 * j:640 * h + 64 * j + 64],
                        start=(h == 0 and j == 0), stop=(h == 1 and j == 8))
            if DEBUG and si == 0:
                nc.sync.dma_start(dbg["d_outT"], outT[:])
            # order in one bank: gram -> G-copy -> svec-sum -> svrow-copy ->
            # transpose -> G-col copy (chained via overlapping regions, since
            # a matmul 'start' resets the whole bank)
            nc.vector.tensor_copy(G_cat[:, 65 * si:65 * si + 64],
                                  gm[0:64, 0:64])
            for h in range(2):
                for j in range(9):
                    nc.tensor.matmul(
                        gm[0:1, 0:64], onesc[:],
                        outT[:, 640 * h + 64 * j:640 * h + 64 * j + 64],
                        start=(h == 0 and j == 0), stop=(h == 1 and j == 8))
            svrow = p2.tile([1, 64], F16, tag="svrow", name="svrow")
            nc.vector.tensor_copy(svrow[:], gm[0:1, 0:64])
            nc.tensor.matmul(gm[0:64, 0:1], svrow[:], onesc[0:1, 0:1],
                             start=True, stop=True)
            nc.vector.tensor_copy(G_cat[:, 65 * si + 64:65 * si + 65],
                                  gm[0:64, 0:1])
            if si == 3:
                # BN partials for first 4 scheduled scales (off critical path)
                for t in range(2):
                    zgp = (pgx if t == 0 else pcv).tile(
                        [128, 512], F32, tag="gx" if t == 0 else "cv",
                        name="zgP")
                    nc.tensor.matmul(zgp[:, 0:260],
                                     zwtf_sb[:, 128 * t:128 * t + 128],
                                     G_cat[:, 0:260], start=True, stop=True)
                    zzp = p2.tile([128, 260], F32, tag="zz", name="zz")
                    nc.vector.tensor_tensor(
                        zzp[:], zgp[:, 0:260],
                        zw65g_sb[:, 325 * t:325 * t + 260],
                        op=AluOpType.mult)
                    s2p = p2.tile([128, 4], F32, tag="s2p", name="s2p")
                    nc.vector.tensor_reduce(
                        s2p[:],
                        zzp[:].rearrange("p (s c) -> p s c", c=65), AXX,
                        AluOpType.add)
                    nc.vector.tensor_copy(arin_sb[:, 10 * t:10 * t + 4],
                                          s2p[:])
                    nc.vector.tensor_copy(
                        arin_sb[:, 10 * t + 5:10 * t + 9],
                        bass.AP(tensor=zgp[:].tensor,
                                offset=zgp[:].offset + 64,
                                ap=[[zgp[:].ap[0][0], 128], [65, 4]]))

        if DEBUG:
            dg = sb.tile([128, 640], F32, tag="dbg4", name="dbg4")
            nc.vector.tensor_copy(dg[:], g_all[0][:, 0:640])
            nc.sync.dma_start(dbg["d_gall"], dg[:])
            nc.sync.dma_start(dbg["d_t16"], t16[:])
            nc.sync.dma_start(dbg["d_p16"], p16_sb[2][:])
            nc.sync.dma_start(dbg["d_Gcat"], G_cat[:])
            nc.sync.dma_start(dbg["d_xw0"], xw_sb[(2, 0)][:])
            nc.sync.dma_start(dbg["d_xw1"], xw_sb[(2, 1)][:])
        # last scheduled scale's BN stats (cols 260:325 of G_cat)
        for t in range(2):
            zg = (pgx if t == 0 else pcv).tile(
                [128, 512], F32, tag="gx" if t == 0 else "cv", name="zgL")
            nc.tensor.matmul(zg[:, 0:65], zwtf_sb[:, 128 * t:128 * t + 128],
                             G_cat[:, 260:325], start=True, stop=True)
            zzl = p2.tile([128, 65], F32, tag="zzL", name="zzL")
            nc.vector.tensor_tensor(zzl[:], zg[:, 0:65],
                                    zw65g_sb[:, 325 * t + 260:325 * t + 325],
                                    op=AluOpType.mult)
            s2l = p2.tile([128, 1], F32, tag="s2l", name="s2l")
            nc.vector.tensor_reduce(s2l[:], zzl[:], AXX, AluOpType.add)
            nc.vector.tensor_copy(arin_sb[:, 10 * t + 4:10 * t + 5], s2l[:])
            nc.vector.tensor_copy(arin_sb[:, 10 * t + 9:10 * t + 10],
                                  zg[:, 64:65])

        if DEBUG:
            nc.sync.dma_start(dbg["d_arin"], arin_sb[:])
        # ---------------- stats AllGather ----------------
        arin = dram.tile([128, 20], F16, name="arin_d")
        arout = dram.tile([128 * NCORES, 20], F16, name="arout_d")
        nc.sync.dma_start(arin[:], arin_sb[:])
        nc.gpsimd.collective_compute(
            "AllGather", AluOpType.bypass,
            replica_groups=[list(range(NCORES))],
            ins=[arin.opt()], outs=[arout.opt()])
        gath = sb.tile([128, 160], F16, tag="gath", name="gath")
        src = bass.AP(tensor=arout[:].tensor, offset=arout[:].offset,
                      ap=[[20, 128], [2560, 8], [1, 20]])
        nc.sync.dma_start(gath[:], src)
        stats = sb.tile([128, 20], F32, tag="stats", name="stats")
        nc.vector.tensor_reduce(
            stats[:],
            bass.AP(tensor=gath[:].tensor, offset=gath[:].offset,
                    ap=[[gath[:].ap[0][0], 128], [1, 20], [20, 8]]),
            AXX, AluOpType.add)

        if DEBUG:
            nc.sync.dma_start(dbg["d_stats"], stats[:])
        # ---------------- BN coefficients (SCHED order) ----------------
        a16 = sb.tile([128, 10], F16, tag="a16", name="a16")
        bacc_t = [sb.tile([128, 1], F32, tag=f"bacc{t}", name=f"bacc{t}")
                  for t in range(2)]
        for t in range(2):
            s2v = stats[:, 10 * t:10 * t + 5]
            s1v = stats[:, 10 * t + 5:10 * t + 10]
            mean = p2.tile([128, 5], F32, tag="mean", name="mean")
            nc.vector.tensor_scalar_mul(mean[:], s1v, 1.0 / NPIX)
            m2 = p2.tile([128, 5], F32, tag="m2", name="m2")
            nc.vector.tensor_tensor(m2[:], mean[:], mean[:], op=AluOpType.mult)
            var = p2.tile([128, 5], F32, tag="var", name="var")
            nc.vector.scalar_tensor_tensor(var[:], s2v, 1.0 / NPIX, m2[:],
                                           op0=AluOpType.mult,
                                           op1=AluOpType.subtract)
            sq = p2.tile([128, 5], F32, tag="sq", name="sq")
            nc.scalar.activation(sq[:], var[:], AFT.Sqrt,
                                 bias=gmp_sb[:, 12:13])
            rinv = p2.tile([128, 5], F32, tag="rinv", name="rinv")
            nc.vector.reciprocal_approx_fast(rinv[:], sq[:])
            af = p2.tile([128, 5], F32, tag="af", name="af")
            nc.vector.tensor_tensor(af[:], rinv[:], gmp_sb[:, 5 * t:5 * t + 5],
                                    op=AluOpType.mult)
            nc.vector.tensor_copy(a16[:, 5 * t:5 * t + 5], af[:])
            tmb = p2.tile([128, 5], F32, tag="tmb", name="tmb")
            nc.vector.tensor_tensor(tmb[:], af[:], mean[:], op=AluOpType.mult)
            tmbr = p2.tile([128, 1], F32, tag="tmbr", name="tmbr")
            nc.vector.tensor_reduce(tmbr[:], tmb[:], AXX, AluOpType.add)
            nc.vector.tensor_tensor(bacc_t[t][:], gmp_sb[:, 10 + t:11 + t],
                                    tmbr[:], op=AluOpType.subtract)
        # a5cat rows via PE transposes: one accumulation group per bank
        # (start zeroes the bank; disjoint-column matmuls add into zeros),
        # then 3 wide copies instead of 10 narrow ones
        a5cat = sb.tile([1, 1280], F16, tag="a5cat", name="a5cat")
        banks = [(pgx, "gx", 0, 4), (pcv, "cv", 4, 8), (psc, "sc", 8, 10)]
        for pool, tag, i0, i1 in banks:
            atp = pool.tile([128, 512], F32, tag=tag, name="tp")
            for ii in range(i0, i1):
                si, t = ii // 2, ii % 2
                nc.tensor.matmul(
                    atp[0:1, 128 * (ii - i0):128 * (ii - i0) + 128],
                    a16[:, 5 * t + si:5 * t + si + 1],
                    i128_sb[:], start=(ii == i0), stop=(ii == i1 - 1))
            nc.vector.tensor_copy(a5cat[0:1, 128 * i0:128 * i1],
                                  atp[0:1, 0:128 * (i1 - i0)])
        W_sb = []
        for si in range(5):
            abp = (pcv if si % 2 else pgx).tile(
                [128, 512], F32, tag="cv" if si % 2 else "gx", name="ab")
            nc.tensor.matmul(abp[:, 0:256], ones128[:],
                             a5cat[0:1, 256 * si:256 * si + 256],
                             start=True, stop=True)
            W = sb.tile([128, 256], F16, tag=f"W{si}", name=f"W{si}")
            nc.vector.tensor_tensor(W[:], zwt16_sb[:], abp[:, 0:256],
                                    op=AluOpType.mult)
            W_sb.append(W)

        if DEBUG:
            nc.sync.dma_start(dbg["d_a16"], a16[:])
            nc.sync.dma_start(dbg["d_W0"], W_sb[0][:])
        # ---------------- final matmul + store ----------------
        for t in range(2):
            out_sb = sb.tile([128, QL], F32, tag=f"osb{t}", name=f"osb{t}")
            for h in range(2):
                for par in range(2):
                    nb = 5 if par == 0 else 4
                    fp = psc.tile([128, 576], F32, tag="sc", name="sc")
                    for si in range(5):
                        nc.tensor.matmul(
                            fp[:, 0:64 * nb],
                            W_sb[si][64 * par:64 * par + 64,
                                     128 * t:128 * t + 128],
                            xw_sb[(SCHED[si], h)][64 * par:64 * par + 64,
                                                  0:64 * nb],
                            start=(si == 0), stop=(si == 4))
                    dst = bass.AP(
                        tensor=out_sb[:].tensor,
                        offset=out_sb[:].offset + 64 * (9 * h + par),
                        ap=[[out_sb[:].ap[0][0], 128], [128, nb], [1, 64]])
                    nc.vector.tensor_scalar_add(
                        dst,
                        fp[:].rearrange("p (b c) -> p b c", c=64)[:, 0:nb, :],
                        bacc_t[t][:])
                nc.sync.dma_start(
                    out_d[:, QL * t + 576 * h:QL * t + 576 * h + 576],
                    out_sb[:, 576 * h:576 * h + 576])

    nc.compile()
    return nc


def kernel(**inputs):
    f32, f16 = np.float32, np.float16
    persp = np.asarray(inputs['perspective'], dtype=f32)
    t_w = np.asarray(inputs['t_w'], dtype=f32)
    z_w = np.asarray(inputs['z_w'], dtype=f32)
    if 'nc' not in _CACHED:
        _CACHED['nc'] = build()
    nc = _CACHED['nc']
    KT = [max(1, c // 128) for c in CR]

    # local query order: col L = 64*j + q  ->  global n = 36*q + 18*h + j
    Lq = np.arange(QL)
    qv, jv = Lq % 64, Lq // 64
    twt16 = np.zeros((128, 128), f16)
    twt = np.ascontiguousarray(t_w.T)
    twt16[:, 0:64] = twt[0:128].astype(f16)
    twt16[:, 64:128] = twt[128:256].astype(f16)
    zwt = np.ascontiguousarray(z_w.T)
    zw65 = np.zeros((128, 650), f32)
    for t in range(2):
        for si in range(5):
            zw65[:, 325 * t + 65 * si:325 * t + 65 * si + 64] = \
                z_w[128 * t:128 * t + 128, :]
    gmp = np.zeros((128, 13), f32)
    for t in range(2):
        for si in range(5):
            gmp[:, 5 * t + si] = np.asarray(
                inputs[f'bn{SCHED[si]}_g'], f32)[128 * t:128 * t + 128]
        gmp[:, 10 + t] = sum(np.asarray(inputs[f'bn{s}_b'], f32)
                             for s in range(5))[128 * t:128 * t + 128]
    gmp[:, 12] = EPS
    i128 = np.eye(128, dtype=f16)
    nkt = sum(KT)
    pwt = np.zeros((128, 64 * nkt), f16)
    gwt = np.zeros((128, 64 * nkt), f16)
    koff = 0
    for s in range(5):
        pw = np.asarray(inputs[f'p{s}_w'], f32).T
        gw = np.asarray(inputs[f'g{s}_w'], f32).T
        for kk in range(KT[s]):
            r0, r1 = 128 * kk, min(128 * kk + 128, CR[s])
            pwt[0:r1 - r0, koff:koff + 64] = pw[r0:r1].astype(f16)
            gwt[0:r1 - r0, koff:koff + 64] = gw[r0:r1].astype(f16)
            koff += 64

    in_maps = []
    for i in range(4):
        for h in range(2):
            nglob = 36 * qv + 18 * h + jv
            pi = persp[i].reshape(CP, 2304)[:, nglob]
            p16 = np.zeros((128, 2 * QL), f16)
            p16[:, 0:QL] = pi[0:128].astype(f16)
            p16[:, QL:] = pi[128:256].astype(f16)
            m = {"persp": p16, "twt": twt16,
                 "zwt16": np.concatenate([zwt, zwt], axis=0).astype(f16),
                 "zwtf": zwt, "zw65": zw65, "gmp": gmp, "i128": i128,
                 "pwt": pwt, "gwt": gwt}
            for s in range(5):
                rs = np.asarray(inputs[f'response{s}'], f32)[i].reshape(CR[s], MS[s])
                rt = np.zeros((min(CR[s], 128), KT[s] * MS[s]), f16)
                for kk in range(KT[s]):
                    r0, r1 = 128 * kk, min(128 * kk + 128, CR[s])
                    rt[0:r1 - r0, MS[s] * kk:MS[s] * kk + MS[s]] = \
                        rs[r0:r1].astype(f16)
                m[f"resp{s}"] = rt
            in_maps.append(m)
    res = bass_utils.run_bass_kernel_spmd(nc, in_maps,
                                          core_ids=list(range(NCORES)))
    _CACHED['res'] = res
    out = np.zeros((4, CP, 2304), np.float32)
    for i in range(4):
        for h in range(2):
            o = res.results[i * 2 + h]["out"]
            full = np.concatenate([o[:, 0:QL], o[:, QL:]], axis=0)
            out[i][:, QL * h:QL * h + QL] = full
    return out.reshape(4, CP, 48, 48)


if __name__ == "__main__":
    from concourse.timeline_sim import TimelineSim
    nc = build()
    tl = TimelineSim(nc, trace=False)
    print(f"TimelineSim: {tl.simulate():.0f} ns")


# revision 44
# speedup vs baseline: 1.0465x; 1.0027x over previous
"""Trainium2 Bass kernel for nn_CNL_5 (5-scale context non-local block).

Sharding: 8 cores = 4 samples x 2 query-subsets. Local query column order is
L = 64*j + q  (q = z-conv input channel = n//36-block, j = n%18), chosen so the
xbar DMA transpose (out[p,b,c] = in[c,128b+p]) directly yields the z-conv
operand x[q, pixel] with j-parity split across partition halves. outT is
padded to 640-col halves so each query-half transposes independently.

BN batch stats travel as per-channel (s1, s2) quadratic forms [128,20] through
one small AllGather; heavy math is fp16/bf16 on the PE at 1 cyc/row.
"""
import numpy as np
import ml_dtypes
from contextlib import ExitStack

import concourse.bass as bass
import concourse.bacc as bacc
import concourse.tile as tile
from concourse import mybir
from concourse import bass_utils
from concourse.alu_op_type import AluOpType

F32 = mybir.dt.float32
F16 = mybir.dt.float16
BF16 = mybir.dt.bfloat16
AFT = mybir.ActivationFunctionType
AXX = mybir.AxisListType.X

NCORES = 8
CP = 256
QL = 1152
CR = [64, 256, 512, 1024, 2048]
MS = [2304, 2304, 576, 144, 36]
CSH = [0.0, 10.0, 15.0, 25.0, 40.0]
EPS = 1e-5
NPIX = 4 * 2304.0
SCHED = [4, 2, 3, 1, 0]
DEBUG = False

_CACHED = {}


def mtiles(M):
    out, off = [], 0
    while off < M:
        w = min(128, M - off)
        out.append((off, w))
        off += w
    return out


def chunks512(N):
    out, off = [], 0
    while off < N:
        w = min(512, N - off)
        out.append((off, w))
        off += w
    return out


def build():
    nc = bacc.Bacc("TRN2", target_bir_lowering=False, debug=False,
                   num_devices=NCORES)
    KT = [len(mtiles(c)) for c in CR]
    persp_d = nc.dram_tensor("persp", [128, 2 * QL], F16, kind="ExternalInput").ap()
    twt_d = nc.dram_tensor("twt", [128, 128], F16, kind="ExternalInput").ap()
    resp_d = [nc.dram_tensor(f"resp{s}", [min(CR[s], 128), KT[s] * MS[s]], F16,
                             kind="ExternalInput").ap() for s in range(5)]
    pwt_d = nc.dram_tensor("pwt", [128, 64 * sum(KT)], F16, kind="ExternalInput").ap()
    gwt_d = nc.dram_tensor("gwt", [128, 64 * sum(KT)], F16, kind="ExternalInput").ap()
    zwt16_d = nc.dram_tensor("zwt16", [128, CP], F16, kind="ExternalInput").ap()
    zwtf_d = nc.dram_tensor("zwtf", [64, CP], F32, kind="ExternalInput").ap()
    zw65_d = nc.dram_tensor("zw65", [128, 650], F32, kind="ExternalInput").ap()
    gmp_d = nc.dram_tensor("gmp", [128, 13], F32, kind="ExternalInput").ap()
    i128_d = nc.dram_tensor("i128", [128, 128], F16, kind="ExternalInput").ap()
    out_d = nc.dram_tensor("out", [128, 2 * QL], F32, kind="ExternalOutput").ap()
    dbg = {}
    if DEBUG:
        for nm, shp, dt in [("d_t16", [64, QL], F16), ("d_p16", [64, MS[2]], F16),
                            ("d_outT", [64, 1280], F16), ("d_xw0", [128, 320], F16),
                            ("d_xw1", [128, 320], F16), ("d_Gcat", [64, 325], F32),
                            ("d_arin", [128, 20], F16), ("d_stats", [128, 20], F32),
                            ("d_a16", [128, 10], F16), ("d_W0", [128, 256], F16),
                            ("d_opn", [64, 576], F32), ("d_opd", [64, 576], F32),
                            ("d_rc", [64, 576], F32), ("d_et", [128, 576], F16),
                            ("d_gall", [128, 640], F32)]:
            dbg[nm] = nc.dram_tensor(nm, shp, dt, kind="ExternalOutput").ap()

    with tile.TileContext(nc) as tc, ExitStack() as ctx:
        sb = ctx.enter_context(tc.tile_pool(name="sb", bufs=1))
        p2 = ctx.enter_context(tc.tile_pool(name="p2", bufs=2))
        et3 = ctx.enter_context(tc.tile_pool(name="et3", bufs=44))
        p16p = ctx.enter_context(tc.tile_pool(name="p16p", bufs=3))
        dram = ctx.enter_context(tc.tile_pool(name="dram", bufs=1, space="DRAM"))
        psc = ctx.enter_context(tc.tile_pool(name="psc", bufs=2, space="PSUM"))
        pop = ctx.enter_context(tc.tile_pool(name="pop", bufs=1, space="PSUM"))
        pcv = ctx.enter_context(tc.tile_pool(name="pcv", bufs=1, space="PSUM"))
        pgx = ctx.enter_context(tc.tile_pool(name="pgx", bufs=1, space="PSUM"))

        # ---------------- static loads ----------------
        twt_sb = sb.tile([128, 128], F16, tag="twt", name="twt")
        nc.sync.dma_start(twt_sb[:], twt_d)
        persp_sb = sb.tile([128, 2 * QL], F16, tag="persp", name="persp")
        nc.sync.dma_start(persp_sb[:], persp_d)
        pwt_sb = sb.tile([128, 64 * sum(KT)], F16, tag="pwt", name="pwt")
        nc.sync.dma_start(pwt_sb[:], pwt_d)
        resp_sb = [None] * 5
        s0_ = SCHED[0]
        r = sb.tile([min(CR[s0_], 128), KT[s0_] * MS[s0_]], F16,
                    tag=f"resp{s0_}", name=f"resp{s0_}")
        nc.sync.dma_start(r[:], resp_d[s0_])
        resp_sb[s0_] = r
        gwt_sb = sb.tile([128, 64 * sum(KT)], F16, tag="gwt", name="gwt")
        nc.sync.dma_start(gwt_sb[:], gwt_d)
        for s in SCHED[1:]:
            r = sb.tile([min(CR[s], 128), KT[s] * MS[s]], F16,
                        tag=f"resp{s}", name=f"resp{s}")
            nc.sync.dma_start(r[:], resp_d[s])
            resp_sb[s] = r
        KOFF = [64 * sum(KT[:s]) for s in range(5)]
        zwt16_sb = sb.tile([128, CP], F16, tag="zwt16", name="zwt16")
        nc.sync.dma_start(zwt16_sb[:], zwt16_d)
        zwtf_sb = sb.tile([64, CP], F32, tag="zwtf", name="zwtf")
        nc.sync.dma_start(zwtf_sb[:], zwtf_d)
        zw65g_sb = sb.tile([128, 650], F32, tag="zw65", name="zw65")
        nc.sync.dma_start(zw65g_sb[:], zw65_d)
        gmp_sb = sb.tile([128, 13], F32, tag="gmp", name="gmp")
        nc.sync.dma_start(gmp_sb[:], gmp_d)
        i128_sb = sb.tile([128, 128], F16, tag="i128", name="i128")
        nc.sync.dma_start(i128_sb[:], i128_d)
        bias_sb = []
        for s in range(5):
            bt = sb.tile([128, 1], F32, tag=f"bias{s}", name=f"bias{s}")
            nc.vector.memset(bt[:], -CSH[s])
            bias_sb.append(bt)
        ones128 = sb.tile([1, 128], F16, tag="ones128", name="ones128")
        nc.vector.memset(ones128[:], 1.0)
        onesc = sb.tile([64, 1], F16, tag="onesc", name="onesc")
        nc.vector.memset(onesc[:], 1.0)
        g_all = [sb.tile([128, 128 * 18], BF16, tag=f"gall{i}", name=f"gall{i}")
                 for i in range(3)]
        for i in range(3):
            nc.gpsimd.memset(
                g_all[i][:].rearrange("p (k c) -> p k c", c=128)[:, :, 64:128], 1.0)
        # outT ring: pre-zero the 64-col pads of both ring slots
        oT = [p2.tile([64, 1280], F16, tag="outT", name=f"outTz{i}")
              for i in range(2)]
        for i in range(2):
            for h in range(2):
                nc.gpsimd.memset(oT[i][:, 640 * h + 576:640 * h + 640], 0.0)

        # ---------------- t conv: t16 [64, QL] ----------------
        t16 = sb.tile([64, QL], F16, tag="t16", name="t16")
        for off, w in chunks512(QL):
            tp = pgx.tile([128, 512], F32, tag="gx", name="gx")
            for kk in range(2):
                nc.tensor.matmul(tp[0:64, 0:w], twt_sb[:, 64 * kk:64 * kk + 64],
                                 persp_sb[:, QL * kk + off:QL * kk + off + w],
                                 start=(kk == 0), stop=(kk == 1))
            nc.vector.tensor_copy(t16[:, off:off + w], tp[0:64, 0:w])

        # ---------------- per-scale p/g convs (as deferrable units) --------
        p16_sb, xw_sb = {}, {}

        def conv_units(s, evac_eng):
            nct = mtiles(CR[s])
            p16 = p16p.tile([64, MS[s]], F16, tag="p16", name=f"p16_{s}")
            p16_sb[s] = p16
            units = []

            def p_unit(off, w):
                def emit():
                    pp = pcv.tile([128, 512], F32, tag="cv", name="cv")
                    for kk, (co, cw) in enumerate(nct):
                        nc.tensor.matmul(
                            pp[0:64, 0:w],
                            pwt_sb[0:cw, KOFF[s] + 64 * kk:KOFF[s] + 64 * kk + 64],
                            resp_sb[s][0:cw, MS[s] * kk + off:MS[s] * kk + off + w],
                            start=(kk == 0), stop=(kk == len(nct) - 1))
                    evac_eng.tensor_copy(p16[:, off:off + w], pp[0:64, 0:w])
                return emit

            def g_unit(b0, batch):
                def emit():
                    ga = g_all[SCHED.index(s) % 3]
                    gp = pcv.tile([128, 512], F32, tag="cv", name="cv")
                    for k, (moff, mw) in enumerate(batch):
                        for kk, (co, cw) in enumerate(nct):
                            nc.tensor.matmul(
                                gp[0:mw, 64 * k:64 * k + 64],
                                resp_sb[s][0:cw, MS[s] * kk + moff:MS[s] * kk + moff + mw],
                                gwt_sb[0:cw, KOFF[s] + 64 * kk:KOFF[s] + 64 * kk + 64],
                                start=(kk == 0), stop=(kk == len(nct) - 1))
                    dst = ga[:].rearrange("p (k c) -> p k c", c=128)[
                        :, b0:b0 + len(batch), 0:64]
                    src = gp[:].rearrange("p (k c) -> p k c", c=64)[
                        :, 0:len(batch), :]
                    evac_eng.tensor_copy(dst, src)
                return emit

            for off, w in chunks512(MS[s]):
                units.append((s, p_unit(off, w)))
            mts = mtiles(MS[s])
            for b0 in range(0, len(mts), 8):
                units.append((s, g_unit(b0, mts[b0:b0 + 8])))
            return units

        for _, u in conv_units(SCHED[0], nc.vector):
            u()
        pending = conv_units(SCHED[1], nc.vector)

        # ---------------- attention per scale ----------------
        arin_sb = sb.tile([128, 20], F16, tag="arin", name="arin")
        G_cat = sb.tile([64, 325], F32, tag="G_cat", name="G_cat")
        for si, s in enumerate(SCHED):
            mts = mtiles(MS[s])
            ga = g_all[si % 3]
            while pending and pending[0][0] == s:
                pending.pop(0)[1]()
            if si + 2 < 5:
                pending += conv_units(SCHED[si + 2], nc.vector)
            outT = p2.tile([64, 1280], F16, tag="outT", name=f"outT{s}")
            gm = pgx.tile([128, 512], F32, tag="gx", name="gx")
            for h in range(2):
                op = pop.tile([128, 576], F32, tag="op", name="op")
                for k, (moff, mw) in enumerate(mts):
                    sc = psc.tile([128, 576], F32, tag="sc", name="sc")
                    for co, cw in ((0, 512), (512, 64)):
                        nc.tensor.matmul(
                            sc[0:mw, co:co + cw],
                            p16_sb[s][:, moff:moff + mw],
                            t16[:, 576 * h + co:576 * h + co + cw],
                            start=True, stop=True)
                    et = et3.tile([128, 576], BF16, tag="et", name="et")
                    nc.scalar.activation(et[0:mw, :], sc[0:mw, :], AFT.Exp,
                                         bias=bias_sb[s][0:mw, :])
                    if DEBUG and si == 0 and h == 0 and k == 0:
                        de = sb.tile([128, 576], F16, tag="dbg3", name="dbg3")
                        nc.vector.tensor_copy(de[:], et[:])
                        nc.sync.dma_start(dbg["d_et"], de[:])
                    for co, cw in ((0, 512), (512, 64)):
                        nc.tensor.matmul(
                            op[:, co:co + cw],
                            ga[0:mw, 128 * k:128 * k + 128],
                            et[0:mw, co:co + cw],
                            start=(k == 0), stop=(k == len(mts) - 1))
                    if pending and (k % 2 == 1 or len(mts) < 10):
                        pending.pop(0)[1]()
                rc = p2.tile([64, 576], F32, tag="rc", name="rc")
                if DEBUG and si == 0 and h == 1:
                    dn = sb.tile([64, 576], F32, tag="dbg1", name="dbg1")
                    nc.vector.tensor_copy(dn[:], op[0:64, :])
                    nc.sync.dma_start(dbg["d_opn"], dn[:])
                    dd = sb.tile([64, 576], F32, tag="dbg2", name="dbg2")
                    nc.vector.tensor_copy(dd[:], op[64:128, :])
                    nc.sync.dma_start(dbg["d_opd"], dd[:])
                nc.vector.reciprocal(rc[:], op[64:128, :])
                if DEBUG and si == 0 and h == 1:
                    nc.sync.dma_start(dbg["d_rc"], rc[:])
                if len(mts) < 10:
                    # small scales: Act idles at the boundary — copy the
                    # numerator out on Act in parallel with the reciprocal so
                    # the op PSUM ring frees ~2x faster for the next half
                    num = p2.tile([64, 576], F32, tag="num", name="num")
                    nc.scalar.activation(num[:], op[0:64, :], AFT.Copy)
                    nc.vector.tensor_tensor(outT[:, 640 * h:640 * h + 576],
                                            num[:], rc[:], op=AluOpType.mult)
                else:
                    nc.vector.tensor_tensor(outT[:, 640 * h:640 * h + 576],
                                            op[0:64, :], rc[:],
                                            op=AluOpType.mult)
                # per-half xbar transpose + svec + gram
                xw = sb.tile([128, 5 * 64], F16, tag=f"xw{s}_{h}",
                             name=f"xw{s}_{h}")
                xw_sb[(s, h)] = xw
                nc.sync.dma_start_transpose(
                    xw[:].rearrange("p (b c) -> p b c", c=64),
                    outT[:, 640 * h:640 * h + 640])
                for j in range(9):
                    nc.tensor.matmul(
                        gm[0:64, 0:64],
                        outT[:, 640 * h + 64 * j:640 * h + 64 * j + 64],
                        outT[:, 640 * h + 64 * j:640 * h + 64 * j + 64],
                        start=(h == 0 and j == 0), stop=(h == 1 and j == 8))
                if si == 4:
                    if h == 0:
                        svt = pcv.tile([128, 512], F32, tag="cv", name="svt")
                    for j in range(9):
                        nc.tensor.matmul(
                            svt[0:1, 0:64], onesc[:],
                            outT[:, 640 * h + 64 * j:640 * h + 64 * j + 64],
                            start=(h == 0 and j == 0),
                            stop=(h == 1 and j == 8))
            if DEBUG and si == 0:
                nc.sync.dma_start(dbg["d_outT"], outT[:])
            # order in one bank: gram -> G-copy -> svec-sum -> svrow-copy ->
            # transpose -> G-col copy (chained via overlapping regions, since
            # a matmul 'start' resets the whole bank)
            nc.vector.tensor_copy(G_cat[:, 65 * si:65 * si + 64],
                                  gm[0:64, 0:64])
            if si == 4:
                # svec already accumulated in pcv (svt); finish there
                svrow = p2.tile([1, 64], F16, tag="svrow", name="svrow")
                nc.vector.tensor_copy(svrow[:], svt[0:1, 0:64])
                nc.tensor.matmul(svt[0:64, 0:1], svrow[:], onesc[0:1, 0:1],
                                 start=True, stop=True)
                nc.vector.tensor_copy(G_cat[:, 65 * si + 64:65 * si + 65],
                                      svt[0:64, 0:1])
            else:
                for h in range(2):
                    for j in range(9):
                        nc.tensor.matmul(
                            gm[0:1, 0:64], onesc[:],
                            outT[:, 640 * h + 64 * j:640 * h + 64 * j + 64],
                            start=(h == 0 and j == 0), stop=(h == 1 and j == 8))
                svrow = p2.tile([1, 64], F16, tag="svrow", name="svrow")
                nc.vector.tensor_copy(svrow[:], gm[0:1, 0:64])
                nc.tensor.matmul(gm[0:64, 0:1], svrow[:], onesc[0:1, 0:1],
                                 start=True, stop=True)
                nc.vector.tensor_copy(G_cat[:, 65 * si + 64:65 * si + 65],
                                      gm[0:64, 0:1])
            if si == 3:
                # BN partials for first 4 scheduled scales (off critical path)
                for t in range(2):
                    zgp = (pgx if t == 0 else pcv).tile(
                        [128, 512], F32, tag="gx" if t == 0 else "cv",
                        name="zgP")
                    nc.tensor.matmul(zgp[:, 0:260],
                                     zwtf_sb[:, 128 * t:128 * t + 128],
                                     G_cat[:, 0:260], start=True, stop=True)
                    zzp = p2.tile([128, 260], F32, tag="zz", name="zz")
                    nc.vector.tensor_tensor(
                        zzp[:], zgp[:, 0:260],
                        zw65g_sb[:, 325 * t:325 * t + 260],
                        op=AluOpType.mult)
                    s2p = p2.tile([128, 4], F32, tag="s2p", name="s2p")
                    nc.vector.tensor_reduce(
                        s2p[:],
                        zzp[:].rearrange("p (s c) -> p s c", c=65), AXX,
                        AluOpType.add)
                    nc.vector.tensor_copy(arin_sb[:, 10 * t:10 * t + 4],
                                          s2p[:])
                    nc.vector.tensor_copy(
                        arin_sb[:, 10 * t + 5:10 * t + 9],
                        bass.AP(tensor=zgp[:].tensor,
                                offset=zgp[:].offset + 64,
                                ap=[[zgp[:].ap[0][0], 128], [65, 4]]))

        if DEBUG:
            dg = sb.tile([128, 640], F32, tag="dbg4", name="dbg4")
            nc.vector.tensor_copy(dg[:], g_all[0][:, 0:640])
            nc.sync.dma_start(dbg["d_gall"], dg[:])
            nc.sync.dma_start(dbg["d_t16"], t16[:])
            nc.sync.dma_start(dbg["d_p16"], p16_sb[2][:])
            nc.sync.dma_start(dbg["d_Gcat"], G_cat[:])
            nc.sync.dma_start(dbg["d_xw0"], xw_sb[(2, 0)][:])
            nc.sync.dma_start(dbg["d_xw1"], xw_sb[(2, 1)][:])
        # last scheduled scale's BN stats (cols 260:325 of G_cat)
        for t in range(2):
            zg = (pgx if t == 0 else pcv).tile(
                [128, 512], F32, tag="gx" if t == 0 else "cv", name="zgL")
            nc.tensor.matmul(zg[:, 0:65], zwtf_sb[:, 128 * t:128 * t + 128],
                             G_cat[:, 260:325], start=True, stop=True)
            zzl = p2.tile([128, 65], F32, tag="zzL", name="zzL")
            nc.vector.tensor_tensor(zzl[:], zg[:, 0:65],
                                    zw65g_sb[:, 325 * t + 260:325 * t + 325],
                                    op=AluOpType.mult)
            s2l = p2.tile([128, 1], F32, tag="s2l", name="s2l")
            nc.vector.tensor_reduce(s2l[:], zzl[:], AXX, AluOpType.add)
            nc.vector.tensor_copy(arin_sb[:, 10 * t + 4:10 * t + 5], s2l[:])
            nc.vector.tensor_copy(arin_sb[:, 10 * t + 9:10 * t + 10],
                                  zg[:, 64:65])

        if DEBUG:
            nc.sync.dma_start(dbg["d_arin"], arin_sb[:])
        # ---------------- stats AllGather ----------------
        arin = dram.tile([128, 20], F16, name="arin_d")
        arout = dram.tile([128 * NCORES, 20], F16, name="arout_d")
        nc.sync.dma_start(arin[:], arin_sb[:])
        nc.gpsimd.collective_compute(
            "AllGather", AluOpType.bypass,
            replica_groups=[list(range(NCORES))],
            ins=[arin.opt()], outs=[arout.opt()])
        gath = sb.tile([128, 160], F16, tag="gath", name="gath")
        src = bass.AP(tensor=arout[:].tensor, offset=arout[:].offset,
                      ap=[[20, 128], [2560, 8], [1, 20]])
        nc.sync.dma_start(gath[:], src)
        stats = sb.tile([128, 20], F32, tag="stats", name="stats")
        nc.vector.tensor_reduce(
            stats[:],
            bass.AP(tensor=gath[:].tensor, offset=gath[:].offset,
                    ap=[[gath[:].ap[0][0], 128], [1, 20], [20, 8]]),
            AXX, AluOpType.add)

        if DEBUG:
            nc.sync.dma_start(dbg["d_stats"], stats[:])
        # ---------------- BN coefficients (SCHED order) ----------------
        a16 = sb.tile([128, 10], F16, tag="a16", name="a16")
        bacc_t = [sb.tile([128, 1], F32, tag=f"bacc{t}", name=f"bacc{t}")
                  for t in range(2)]
        for t in range(2):
            s2v = stats[:, 10 * t:10 * t + 5]
            s1v = stats[:, 10 * t + 5:10 * t + 10]
            mean = p2.tile([128, 5], F32, tag="mean", name="mean")
            nc.vector.tensor_scalar_mul(mean[:], s1v, 1.0 / NPIX)
            m2 = p2.tile([128, 5], F32, tag="m2", name="m2")
            nc.vector.tensor_tensor(m2[:], mean[:], mean[:], op=AluOpType.mult)
            var = p2.tile([128, 5], F32, tag="var", name="var")
            nc.vector.scalar_tensor_tensor(var[:], s2v, 1.0 / NPIX, m2[:],
                                           op0=AluOpType.mult,
                                           op1=AluOpType.subtract)
            sq = p2.tile([128, 5], F32, tag="sq", name="sq")
            nc.scalar.activation(sq[:], var[:], AFT.Sqrt,
                                 bias=gmp_sb[:, 12:13])
            rinv = p2.tile([128, 5], F32, tag="rinv", name="rinv")
            nc.vector.reciprocal_approx_fast(rinv[:], sq[:])
            af = p2.tile([128, 5], F32, tag="af", name="af")
            nc.vector.tensor_tensor(af[:], rinv[:], gmp_sb[:, 5 * t:5 * t + 5],
                                    op=AluOpType.mult)
            nc.vector.tensor_copy(a16[:, 5 * t:5 * t + 5], af[:])
            tmb = p2.tile([128, 5], F32, tag="tmb", name="tmb")
            nc.vector.tensor_tensor(tmb[:], af[:], mean[:], op=AluOpType.mult)
            tmbr = p2.tile([128, 1], F32, tag="tmbr", name="tmbr")
            nc.vector.tensor_reduce(tmbr[:], tmb[:], AXX, AluOpType.add)
            nc.vector.tensor_tensor(bacc_t[t][:], gmp_sb[:, 10 + t:11 + t],
                                    tmbr[:], op=AluOpType.subtract)
        # a5cat rows via PE transposes: one accumulation group per bank
        # (start zeroes the bank; disjoint-column matmuls add into zeros),
        # then 3 wide copies instead of 10 narrow ones
        a5cat = sb.tile([1, 1280], F16, tag="a5cat", name="a5cat")
        banks = [(pgx, "gx", 0, 4), (pcv, "cv", 4, 8), (psc, "sc", 8, 10)]
        for pool, tag, i0, i1 in banks:
            atp = pool.tile([128, 512], F32, tag=tag, name="tp")
            for ii in range(i0, i1):
                si, t = ii // 2, ii % 2
                nc.tensor.matmul(
                    atp[0:1, 128 * (ii - i0):128 * (ii - i0) + 128],
                    a16[:, 5 * t + si:5 * t + si + 1],
                    i128_sb[:], start=(ii == i0), stop=(ii == i1 - 1))
            nc.vector.tensor_copy(a5cat[0:1, 128 * i0:128 * i1],
                                  atp[0:1, 0:128 * (i1 - i0)])
        W_sb = []
        for si in range(5):
            abp = (pcv if si % 2 else pgx).tile(
                [128, 512], F32, tag="cv" if si % 2 else "gx", name="ab")
            nc.tensor.matmul(abp[:, 0:256], ones128[:],
                             a5cat[0:1, 256 * si:256 * si + 256],
                             start=True, stop=True)
            W = sb.tile([128, 256], F16, tag=f"W{si}", name=f"W{si}")
            nc.vector.tensor_tensor(W[:], zwt16_sb[:], abp[:, 0:256],
                                    op=AluOpType.mult)
            W_sb.append(W)

        if DEBUG:
            nc.sync.dma_start(dbg["d_a16"], a16[:])
            nc.sync.dma_start(dbg["d_W0"], W_sb[0][:])
        # ---------------- final matmul + store ----------------
        for t in range(2):
            out_sb = sb.tile([128, QL], F32, tag=f"osb{t}", name=f"osb{t}")
            for h in range(2):
                for par in range(2):
                    nb = 5 if par == 0 else 4
                    fp = psc.tile([128, 576], F32, tag="sc", name="sc")
                    for si in range(5):
                        nc.tensor.matmul(
                            fp[:, 0:64 * nb],
                            W_sb[si][64 * par:64 * par + 64,
                                     128 * t:128 * t + 128],
                            xw_sb[(SCHED[si], h)][64 * par:64 * par + 64,
                                                  0:64 * nb],
                            start=(si == 0), stop=(si == 4))
                    dst = bass.AP(
                        tensor=out_sb[:].tensor,
                        offset=out_sb[:].offset + 64 * (9 * h + par),
                        ap=[[out_sb[:].ap[0][0], 128], [128, nb], [1, 64]])
                    nc.vector.tensor_scalar_add(
                        dst,
                        fp[:].rearrange("p (b c) -> p b c", c=64)[:, 0:nb, :],
                        bacc_t[t][:])
                nc.sync.dma_start(
                    out_d[:, QL * t + 576 * h:QL * t + 576 * h + 576],
                    out_sb[:, 576 * h:576 * h + 576])

    nc.compile()
    return nc


def kernel(**inputs):
    f32, f16 = np.float32, np.float16
    persp = np.asarray(inputs['perspective'], dtype=f32)
    t_w = np.asarray(inputs['t_w'], dtype=f32)
    z_w = np.asarray(inputs['z_w'], dtype=f32)
    if 'nc' not in _CACHED:
        _CACHED['nc'] = build()
    nc = _CACHED['nc']
    KT = [max(1, c // 128) for c in CR]

    # local query order: col L = 64*j + q  ->  global n = 36*q + 18*h + j
    Lq = np.arange(QL)
    qv, jv = Lq % 64, Lq // 64
    twt16 = np.zeros((128, 128), f16)
    twt = np.ascontiguousarray(t_w.T)
    twt16[:, 0:64] = twt[0:128].astype(f16)
    twt16[:, 64:128] = twt[128:256].astype(f16)
    zwt = np.ascontiguousarray(z_w.T)
    zw65 = np.zeros((128, 650), f32)
    for t in range(2):
        for si in range(5):
            zw65[:, 325 * t + 65 * si:325 * t + 65 * si + 64] = \
                z_w[128 * t:128 * t + 128, :]
    gmp = np.zeros((128, 13), f32)
    for t in range(2):
        for si in range(5):
            gmp[:, 5 * t + si] = np.asarray(
                inputs[f'bn{SCHED[si]}_g'], f32)[128 * t:128 * t + 128]
        gmp[:, 10 + t] = sum(np.asarray(inputs[f'bn{s}_b'], f32)
                             for s in range(5))[128 * t:128 * t + 128]
    gmp[:, 12] = EPS
    i128 = np.eye(128, dtype=f16)
    nkt = sum(KT)
    pwt = np.zeros((128, 64 * nkt), f16)
    gwt = np.zeros((128, 64 * nkt), f16)
    koff = 0
    for s in range(5):
        pw = np.asarray(inputs[f'p{s}_w'], f32).T
        gw = np.asarray(inputs[f'g{s}_w'], f32).T
        for kk in range(KT[s]):
            r0, r1 = 128 * kk, min(128 * kk + 128, CR[s])
            pwt[0:r1 - r0, koff:koff + 64] = pw[r0:r1].astype(f16)
            gwt[0:r1 - r0, koff:koff + 64] = gw[r0:r1].astype(f16)
            koff += 64

    in_maps = []
    for i in range(4):
        for h in range(2):
            nglob = 36 * qv + 18 * h + jv
            pi = persp[i].reshape(CP, 2304)[:, nglob]
            p16 = np.zeros((128, 2 * QL), f16)
            p16[:, 0:QL] = pi[0:128].astype(f16)
            p16[:, QL:] = pi[128:256].astype(f16)
            m = {"persp": p16, "twt": twt16,
                 "zwt16": np.concatenate([zwt, zwt], axis=0).astype(f16),
                 "zwtf": zwt, "zw65": zw65, "gmp": gmp, "i128": i128,
                 "pwt": pwt, "gwt": gwt}
            for s in range(5):
                rs = np.asarray(inputs[f'response{s}'], f32)[i].reshape(CR[s], MS[s])
                rt = np.zeros((min(CR[s], 128), KT[s] * MS[s]), f16)
                for kk in range(KT[s]):
                    r0, r1 = 128 * kk, min(128 * kk + 128, CR[s])
                    rt[0:r1 - r0, MS[s] * kk:MS[s] * kk + MS[s]] = \
                        rs[r0:r1].astype(f16)
                m[f"resp{s}"] = rt
            in_maps.append(m)
    res = bass_utils.run_bass_kernel_spmd(nc, in_maps,
                                          core_ids=list(range(NCORES)))
    _CACHED['res'] = res
    out = np.zeros((4, CP, 2304), np.float32)
    for i in range(4):
        for h in range(2):
            o = res.results[i * 2 + h]["out"]
            full = np.concatenate([o[:, 0:QL], o[:, QL:]], axis=0)
            out[i][:, QL * h:QL * h + QL] = full
    return out.reshape(4, CP, 48, 48)


if __name__ == "__main__":
    from concourse.timeline_sim import TimelineSim
    nc = build()
    tl = TimelineSim(nc, trace=False)
    print(f"TimelineSim: {tl.simulate():.0f} ns")
